# revision 9
# baseline (speedup 1.0000x reference)
"""Trainium2 Bass kernel for Bil_layer: 2x bilateral(3x3) + 2x median(3x3).

Data parallel: 2 images x 3 channels per core across 8 cores. Layout per
512x512 plane: 128 partitions x 4 rows; padded tile [128, 3ch, 6, 514]
holds rows -1..4 and cols -1..512 (reflect halos).

Bilateral uses the mirror-weight identity w_{2-dy,2-dx}(y,x) =
w_{dy,dx}(y+dy',x+dx') (dy'=1-dy, dx'=1-dx), so only 4 of 8 tap weights
are computed; mirrored taps read shifted views. Weighted sums accumulate
in PSUM fp32 via PE identity matmuls. Median runs row-phase first
(consuming the input tile's halos directly), then column-phase on free-dim
shifted views -- no intermediate halo exchange.
"""
import numpy as np
from contextlib import ExitStack

import concourse.tile as tile
from concourse import bacc, mybir
from concourse.bass_utils import run_bass_kernel_spmd

P = 128
RR = 4            # data rows per partition
H6 = 6            # padded rows (-1..4)
W = 512
WP = 514          # padded cols (-1..512)
NCH = 3
NIMG = 2
N_CORES = 8

SIGMA_COLOR = 0.1
COLOR2 = 0.01
SIGMA_SPACE = 10.0

F16 = mybir.dt.float16
F32 = mybir.dt.float32
OP = mybir.AluOpType
AF = mybir.ActivationFunctionType


def _gauss1():
    ax = np.arange(3, dtype=np.float64) - 1.0
    g = np.exp(-0.5 * (ax / SIGMA_SPACE) ** 2)
    return g / g.sum()


GO = _gauss1()
GC = float(GO[1] * GO[1])                 # center weight
LNG_C = float(np.log(GO[0] * GO[0]))      # ln g for taps (0,0),(0,2)
LNG_E = float(np.log(GO[0] * GO[1]))      # ln g for taps (0,1),(1,0)
TAPS = [(0, 0), (0, 1), (0, 2), (1, 0)]   # first-half taps; mirror shift = (1-dy, 1-dx)
LNG = [LNG_C, LNG_E, LNG_C, LNG_E]
SCALE1 = float(-0.5 / SIGMA_COLOR ** 2)
SCALE2 = float(-0.5 / COLOR2 ** 2)


def _register_consts(nc):
    for v in sorted({LNG_C, LNG_E}):
        if (F32, v) in nc.const_aps.aps:
            continue
        t = nc.alloc_sbuf_tensor(f"const-f32-{abs(hash(v))}", [P, 1], F32)
        nc.gpsimd.memset(t.ap(), v)
        nc.const_aps.aps[(F32, v)] = t.ap()
    nc.all_engine_barrier()


def _halo_x(nc, X):
    """Fill halos of padded tile X [P, NCH, 6, 514] whose interior
    (rows 1..4, cols 1..512) was written. Col reflect copies first, then
    full-width row halos (so corners ride along)."""
    nc.gpsimd.tensor_copy(out=X[:, :, 1:5, 0:1], in_=X[:, :, 1:5, 2:3])
    nc.gpsimd.tensor_copy(out=X[:, :, 1:5, WP - 1:WP], in_=X[:, :, 1:5, WP - 3:WP - 2])
    nc.gpsimd.dma_start(out=X[0:1, :, 0:1, :], in_=X[0:1, :, 2:3, :])
    nc.gpsimd.dma_start(out=X[P - 1:P, :, 5:6, :], in_=X[P - 1:P, :, 3:4, :])
    nc.sync.dma_start(out=X[1:P, :, 0:1, :], in_=X[0:P - 1, :, 4:5, :])
    nc.sync.dma_start(out=X[0:P - 1, :, 5:6, :], in_=X[1:P, :, 1:2, :])


def _bilateral(nc, pools, Xin, Xout, scale, idt, gid, ones):
    px, pw, pd, pprod, ppsum, psmall = pools

    # --- per-tap color weights (4 first-half taps) ---
    Wt = [None] * 4
    for emi, ki in enumerate((0, 2, 1, 3)):
        dy, dx = TAPS[ki]
        d = pd.tile([P, NCH, RR, W], F16, name="t", tag="d", bufs=2)
        eng = nc.vector if emi < 2 else nc.gpsimd
        eng.tensor_tensor(d[:], Xin[:, :, dy:dy + RR, dx:dx + W],
                          Xin[:, :, 1:5, 1:1 + W], op=OP.subtract)
        nc.scalar.activation(out=d[:], in_=d[:], func=AF.Abs)
        eng.tensor_tensor(d[:, 0], d[:, 0], d[:, 1], op=OP.add)
        eng.tensor_tensor(d[:, 0], d[:, 0], d[:, 2], op=OP.add)
        nc.scalar.activation(out=d[:, 0], in_=d[:, 0], func=AF.Square)
        wk = pw.tile([P, RR + 1, WP], F16, name="t", tag=f"w{ki}", bufs=2)
        nc.scalar.activation(out=wk[:, 0:RR, 1:1 + W], in_=d[:, 0], func=AF.Exp,
                             bias=LNG[ki], scale=scale)
        Wt[ki] = wk

    # --- W halo fixups (mirror taps read shifted views) ---
    # col halos: W00 right <- W02 col x=510; W02 left <- W00 col x=1; W10 right <- own x=511
    nc.gpsimd.tensor_copy(out=Wt[0][:, 0:RR, WP - 1:WP], in_=Wt[2][:, 0:RR, W - 1:W])
    nc.gpsimd.tensor_copy(out=Wt[2][:, 0:RR, 0:1], in_=Wt[0][:, 0:RR, 2:3])
    nc.gpsimd.tensor_copy(out=Wt[3][:, 0:RR, WP - 1:WP], in_=Wt[3][:, 0:RR, WP - 2:WP - 1])
    # bottom halo rows (taps with mirror dy-shift): interior partitions
    for k in (0, 1, 2):
        nc.sync.dma_start(out=Wt[k][0:P - 1, RR:RR + 1, :], in_=Wt[k][1:P, 0:1, :])
    # last partition bottom rows via the reflect partner plane
    nc.gpsimd.dma_start(out=Wt[0][P - 1:P, RR:RR + 1, 2:WP],
                        in_=Wt[2][P - 1:P, RR - 1:RR, 1:WP - 1])
    nc.gpsimd.dma_start(out=Wt[1][P - 1:P, RR:RR + 1, 1:WP - 1],
                        in_=Wt[1][P - 1:P, RR - 1:RR, 1:WP - 1])
    nc.gpsimd.dma_start(out=Wt[2][P - 1:P, RR:RR + 1, 0:W],
                        in_=Wt[0][P - 1:P, RR - 1:RR, 1:WP - 1])

    # --- per-row products + PE accumulation + normalize ---
    for r in range(RR):
        pr = pprod.tile([P, 8, NCH, W], F16, name="t", tag="prod", bufs=2)
        for ki, (dy, dx) in enumerate(TAPS):
            wQ = Wt[ki][:, r:r + 1, 1:1 + W].broadcast_to((P, NCH, W))
            nc.vector.tensor_tensor(pr[:, ki], Xin[:, :, dy + r, dx:dx + W], wQ,
                                    op=OP.mult)
        for ki, (dy, dx) in enumerate(TAPS):
            sy, sx = 1 - dy, 1 - dx
            if ki == 0:
                # split off the last column: it reads W00's right halo (from
                # W02), which would stall the whole mul on tap 2's exp
                wZ = Wt[ki][:, r + sy:r + sy + 1, 2:1 + W].broadcast_to((P, NCH, W - 1))
                nc.vector.tensor_tensor(pr[:, 4 + ki, :, 0:W - 1],
                                        Xin[:, :, r + sy + 1, 2:1 + W], wZ, op=OP.mult)
                wZl = Wt[ki][:, r + sy:r + sy + 1, 1 + W:2 + W].broadcast_to((P, NCH, 1))
                nc.vector.tensor_tensor(pr[:, 4 + ki, :, W - 1:W],
                                        Xin[:, :, r + sy + 1, 1 + W:2 + W], wZl, op=OP.mult)
                continue
            wZ = Wt[ki][:, r + sy:r + sy + 1, 1 + sx:1 + sx + W].broadcast_to((P, NCH, W))
            nc.vector.tensor_tensor(pr[:, 4 + ki], Xin[:, :, r + sy + 1, 1 + sx:1 + sx + W],
                                    wZ, op=OP.mult)
        dn = ppsum.tile([P, W], F32, name="t", tag="ps", bufs=8)
        nc.tensor.matmul(dn[:], gid[:], ones[:], start=True, stop=False)
        for ki, (dy, dx) in enumerate(TAPS):
            sy, sx = 1 - dy, 1 - dx
            nc.tensor.matmul(dn[:], idt[:], Wt[ki][:, r, 1:1 + W],
                             start=False, stop=False)
            nc.tensor.matmul(dn[:], idt[:], Wt[ki][:, r + sy, 1 + sx:1 + sx + W],
                             start=False, stop=(ki == 3))
        rec = psmall.tile([P, W], F32, name="t", tag="rec", bufs=2)
        nc.vector.reciprocal_approx_fast(out=rec[:], in_=dn[:])
        for c in range(NCH):
            t = ppsum.tile([P, W], F32, name="t", tag="ps", bufs=8)
            nc.tensor.matmul(t[:], gid[:], Xin[:, c, r + 1, 1:1 + W],
                             start=True, stop=False)
            for ki in range(4):
                nc.tensor.matmul(t[:], idt[:], pr[:, ki, c], start=False, stop=False)
                nc.tensor.matmul(t[:], idt[:], pr[:, 4 + ki, c],
                                 start=False, stop=(ki == 3))
            nc.vector.tensor_tensor(Xout[:, c, r + 1, 1:1 + W], t[:], rec[:], op=OP.mult)
    _halo_x(nc, Xout)


def _median(nc, pools, Xin, Xout, yview=None):
    """3x3 median: row-phase (lo/med/hi over 3-row windows, consuming Xin's
    halos) then col-phase on free-dim shifted views. If yview is given the
    result rows are DMAed there instead of written to Xout."""
    px, pw, pd, pprod, ppsum, psmall = pools
    mn = lambda o, a, b: nc.vector.tensor_tensor(o, a, b, op=OP.min)
    mx = lambda o, a, b: nc.vector.tensor_max(o, a, b)
    for r in range(RR):
        R0, R1, R2 = Xin[:, :, r], Xin[:, :, r + 1], Xin[:, :, r + 2]
        t1 = psmall.tile([P, NCH, WP], F16, name="t", tag="m1", bufs=2)
        t2 = psmall.tile([P, NCH, WP], F16, name="t", tag="m2", bufs=2)
        lo = psmall.tile([P, NCH, WP], F16, name="t", tag="m3", bufs=1)
        hi = psmall.tile([P, NCH, WP], F16, name="t", tag="m4", bufs=1)
        mn(t1[:], R0, R1)
        mx(t2[:], R0, R1)
        mn(lo[:], t1[:], R2)
        mx(hi[:], t2[:], R2)
        mn(t2[:], t2[:], R2)          # med partial
        mx(t1[:], t1[:], t2[:])       # t1 = med3 of rows
        m = t1
        AS, BS, CS = slice(0, W), slice(1, 1 + W), slice(2, 2 + W)
        H = psmall.tile([P, NCH, W], F16, name="t", tag="n1", bufs=1)
        L = psmall.tile([P, NCH, W], F16, name="t", tag="n2", bufs=1)
        u1 = psmall.tile([P, NCH, W], F16, name="t", tag="n3", bufs=1)
        u2 = psmall.tile([P, NCH, W], F16, name="t", tag="n4", bufs=1)
        v1 = psmall.tile([P, NCH, W], F16, name="t", tag="n4", bufs=1)
        mn(H[:], hi[:, :, AS], hi[:, :, BS])
        mn(H[:], H[:], hi[:, :, CS])
        mx(L[:], lo[:, :, AS], lo[:, :, BS])
        mx(L[:], L[:], lo[:, :, CS])
        mn(u1[:], m[:, :, AS], m[:, :, BS])
        mx(u2[:], m[:, :, AS], m[:, :, BS])
        mn(u2[:], u2[:], m[:, :, CS])
        mx(u1[:], u1[:], u2[:])       # u1 = M
        mn(v1[:], H[:], u1[:])
        mx(H[:], H[:], u1[:])
        mn(H[:], H[:], L[:])
        if yview is not None:
            orow = psmall.tile([P, NCH, W], F16, name="t", tag="or", bufs=1)
            mx(orow[:], v1[:], H[:])
            nc.sync.dma_start(out=yview[:, :, r:r + 1, :], in_=orow[:].unsqueeze(2))
        else:
            mx(Xout[:, :, r + 1, 1:1 + W], v1[:], H[:])
    if yview is None:
        _halo_x(nc, Xout)


def build():
    nc = bacc.Bacc("TRN2", target_bir_lowering=False, debug=False)
    _register_consts(nc)
    xin = nc.dram_tensor("xin", [NIMG, P, NCH, H6, WP], F16, kind="ExternalInput").ap()
    idg = nc.dram_tensor("idg", [2, P, P], F16, kind="ExternalInput").ap()
    yout = nc.dram_tensor("yout", [NIMG, P, NCH, RR, W], F16, kind="ExternalOutput").ap()

    with tile.TileContext(nc) as tc, ExitStack() as ctx:
        px = ctx.enter_context(tc.tile_pool(name="px", bufs=1))
        pw = ctx.enter_context(tc.tile_pool(name="pw", bufs=1))
        pd = ctx.enter_context(tc.tile_pool(name="pd", bufs=1))
        pprod = ctx.enter_context(tc.tile_pool(name="pprod", bufs=1))
        ppsum = ctx.enter_context(tc.psum_pool(name="ppsum", bufs=1))
        psmall = ctx.enter_context(tc.tile_pool(name="psmall", bufs=1))
        pools = (px, pw, pd, pprod, ppsum, psmall)

        idt = psmall.tile([P, P], F16, name="t", tag="id", bufs=1)
        nc.sync.dma_start(out=idt[:], in_=idg[0])
        gid = psmall.tile([P, P], F16, name="t", tag="gid", bufs=1)
        nc.sync.dma_start(out=gid[:], in_=idg[1])
        ones = psmall.tile([P, W], F16, name="t", tag="ones", bufs=1)
        nc.gpsimd.memset(ones[:], 1.0)

        X = []
        for img in range(NIMG):
            xt = px.tile([P, NCH, H6, WP], F16, name="t", tag="x", bufs=3)
            nc.gpsimd.dma_start(out=xt[:], in_=xin[img])
            X.append(xt)
        for img in range(NIMG):
            o = px.tile([P, NCH, H6, WP], F16, name="t", tag="x", bufs=3)
            _bilateral(nc, pools, X[img], o, SCALE1, idt, gid, ones)
            X[img] = o
        o = px.tile([P, NCH, H6, WP], F16, name="t", tag="x", bufs=3)
        _bilateral(nc, pools, X[0], o, SCALE2, idt, gid, ones)
        X[0] = o
        o = px.tile([P, NCH, H6, WP], F16, name="t", tag="x", bufs=3)
        _median(nc, pools, X[0], o)
        m0 = o
        o = px.tile([P, NCH, H6, WP], F16, name="t", tag="x", bufs=3)
        _bilateral(nc, pools, X[1], o, SCALE2, idt, gid, ones)
        X[1] = o
        _median(nc, pools, m0, None, yview=yout[0])
        o = px.tile([P, NCH, H6, WP], F16, name="t", tag="x", bufs=3)
        _median(nc, pools, X[1], o)
        X[1] = o
        _median(nc, pools, X[1], None, yview=yout[1])

    nc.compile()
    return nc


_NC_CACHE = None


def _get_nc():
    global _NC_CACHE
    if _NC_CACHE is None:
        _NC_CACHE = build()
    return _NC_CACHE


def _prep_inputs(x):
    xpad = np.pad(x, ((0, 0), (0, 0), (1, 1), (1, 1)), mode="reflect")
    rows = np.arange(P)[:, None] * RR + np.arange(H6)[None, :]
    win = xpad[:, :, rows, :]                          # (16,3,128,6,514)
    win = win.transpose(0, 2, 1, 3, 4).astype(np.float16)
    return np.ascontiguousarray(win.reshape(N_CORES, NIMG, P, NCH, H6, WP))


def kernel(x):
    x = np.ascontiguousarray(np.asarray(x), dtype=np.float32)
    assert x.shape == (16, 3, 512, 512)
    nc = _get_nc()
    xin = _prep_inputs(x)
    idg = np.ascontiguousarray(
        np.stack([np.eye(P), GC * np.eye(P)]).astype(np.float16))
    in_maps = [{"xin": xin[c], "idg": idg} for c in range(N_CORES)]
    res = run_bass_kernel_spmd(nc, in_maps, list(range(N_CORES)))
    out = np.empty((16, 3, 512, 512), np.float32)
    for c in range(N_CORES):
        y = res.results[c]["yout"]                     # (2,128,3,4,512) f16
        out[2 * c:2 * c + 2] = (y.transpose(0, 2, 1, 3, 4)
                                 .reshape(NIMG, NCH, 512, 512).astype(np.float32))
    return out


# revision 17
# speedup vs baseline: 1.0412x; 1.0412x over previous
"""Trainium2 Bass kernel for Bil_layer: 2x bilateral(3x3) + 2x median(3x3).

Data parallel: 2 images x 3 channels per core across 8 cores. Layout per
512x512 plane: 128 partitions x 4 rows; padded tile [128, 3ch, 6, 514]
holds rows -1..4 and cols -1..512 (reflect halos).

Bilateral uses the mirror-weight identity w_{2-dy,2-dx}(y,x) =
w_{dy,dx}(y+dy',x+dx') (dy'=1-dy, dx'=1-dx), so only 4 of 8 tap weights
are computed; mirrored taps read shifted views. Weighted sums accumulate
in PSUM fp32 via PE identity matmuls. Median runs row-phase first
(consuming the input tile's halos directly), then column-phase on free-dim
shifted views -- no intermediate halo exchange.
"""
import numpy as np
from contextlib import ExitStack

import concourse.tile as tile
from concourse import bacc, mybir
from concourse.bass_utils import run_bass_kernel_spmd

P = 128
RR = 4            # data rows per partition
H6 = 6            # padded rows (-1..4)
W = 512
WP = 514          # padded cols (-1..512)
NCH = 3
NIMG = 2
N_CORES = 8

SIGMA_COLOR = 0.1
COLOR2 = 0.01
SIGMA_SPACE = 10.0

F16 = mybir.dt.float16
F32 = mybir.dt.float32
OP = mybir.AluOpType
AF = mybir.ActivationFunctionType


def _gauss1():
    ax = np.arange(3, dtype=np.float64) - 1.0
    g = np.exp(-0.5 * (ax / SIGMA_SPACE) ** 2)
    return g / g.sum()


GO = _gauss1()
GC = float(GO[1] * GO[1])                 # center weight
LNG_C = float(np.log(GO[0] * GO[0]))      # ln g for taps (0,0),(0,2)
LNG_E = float(np.log(GO[0] * GO[1]))      # ln g for taps (0,1),(1,0)
TAPS = [(0, 0), (0, 1), (0, 2), (1, 0)]   # first-half taps; mirror shift = (1-dy, 1-dx)
LNG = [LNG_C, LNG_E, LNG_C, LNG_E]
SCALE1 = float(-0.5 / SIGMA_COLOR ** 2)
SCALE2 = float(-0.5 / COLOR2 ** 2)


def _register_consts(nc):
    for v in sorted({LNG_C, LNG_E}):
        if (F32, v) in nc.const_aps.aps:
            continue
        t = nc.alloc_sbuf_tensor(f"const-f32-{abs(hash(v))}", [P, 1], F32)
        nc.gpsimd.memset(t.ap(), v)
        nc.const_aps.aps[(F32, v)] = t.ap()
    nc.all_engine_barrier()


def _halo_x(nc, X):
    """Fill halos of padded tile X [P, NCH, 6, 514] whose interior
    (rows 1..4, cols 1..512) was written. Col reflect copies first, then
    full-width row halos (so corners ride along)."""
    nc.gpsimd.tensor_copy(out=X[:, :, 1:5, 0:1], in_=X[:, :, 1:5, 2:3])
    nc.gpsimd.tensor_copy(out=X[:, :, 1:5, WP - 1:WP], in_=X[:, :, 1:5, WP - 3:WP - 2])
    nc.gpsimd.dma_start(out=X[0:1, :, 0:1, :], in_=X[0:1, :, 2:3, :])
    nc.gpsimd.dma_start(out=X[P - 1:P, :, 5:6, :], in_=X[P - 1:P, :, 3:4, :])
    nc.sync.dma_start(out=X[1:P, :, 0:1, :], in_=X[0:P - 1, :, 4:5, :])
    nc.sync.dma_start(out=X[0:P - 1, :, 5:6, :], in_=X[1:P, :, 1:2, :])


def _bilateral(nc, pools, Xin, Xout, scale, idt, gid, ones):
    px, pw, pd, pprod, ppsum, psmall = pools

    # --- per-tap color weights (4 first-half taps) ---
    Wt = [None] * 4
    for emi, ki in enumerate((0, 2, 1, 3)):
        dy, dx = TAPS[ki]
        d = pd.tile([P, NCH, RR, W], F16, name="t", tag="d", bufs=4)
        eng = nc.vector if emi < 2 else nc.gpsimd
        eng.tensor_tensor(d[:], Xin[:, :, dy:dy + RR, dx:dx + W],
                          Xin[:, :, 1:5, 1:1 + W], op=OP.subtract)
        nc.scalar.activation(out=d[:], in_=d[:], func=AF.Abs)
        eng.tensor_tensor(d[:, 0], d[:, 0], d[:, 1], op=OP.add)
        eng.tensor_tensor(d[:, 0], d[:, 0], d[:, 2], op=OP.add)
        nc.scalar.activation(out=d[:, 0], in_=d[:, 0], func=AF.Square)
        wk = pw.tile([P, RR + 1, WP], F16, name="t", tag=f"w{ki}", bufs=2)
        nc.scalar.activation(out=wk[:, 0:RR, 1:1 + W], in_=d[:, 0], func=AF.Exp,
                             bias=LNG[ki], scale=scale)
        Wt[ki] = wk

    # --- W halo fixups (mirror taps read shifted views) ---
    # col halos: W00 right <- W02 col x=510; W02 left <- W00 col x=1; W10 right <- own x=511
    nc.gpsimd.tensor_copy(out=Wt[0][:, 0:RR, WP - 1:WP], in_=Wt[2][:, 0:RR, W - 1:W])
    nc.gpsimd.tensor_copy(out=Wt[2][:, 0:RR, 0:1], in_=Wt[0][:, 0:RR, 2:3])
    nc.gpsimd.tensor_copy(out=Wt[3][:, 0:RR, WP - 1:WP], in_=Wt[3][:, 0:RR, WP - 2:WP - 1])
    # bottom halo rows (taps with mirror dy-shift): interior partitions
    for k in (0, 1, 2):
        nc.sync.dma_start(out=Wt[k][0:P - 1, RR:RR + 1, :], in_=Wt[k][1:P, 0:1, :])
    # last partition bottom rows via the reflect partner plane
    nc.gpsimd.dma_start(out=Wt[0][P - 1:P, RR:RR + 1, 2:WP],
                        in_=Wt[2][P - 1:P, RR - 1:RR, 1:WP - 1])
    nc.gpsimd.dma_start(out=Wt[1][P - 1:P, RR:RR + 1, 1:WP - 1],
                        in_=Wt[1][P - 1:P, RR - 1:RR, 1:WP - 1])
    nc.gpsimd.dma_start(out=Wt[2][P - 1:P, RR:RR + 1, 0:W],
                        in_=Wt[0][P - 1:P, RR - 1:RR, 1:WP - 1])

    # --- per-row products + PE accumulation + normalize ---
    for r in range(RR):
        pr = pprod.tile([P, 8, NCH, W], F16, name="t", tag="prod", bufs=1)
        for ki, (dy, dx) in enumerate(TAPS):
            wQ = Wt[ki][:, r:r + 1, 1:1 + W].broadcast_to((P, NCH, W))
            nc.vector.tensor_tensor(pr[:, ki], Xin[:, :, dy + r, dx:dx + W], wQ,
                                    op=OP.mult)
        for ki, (dy, dx) in enumerate(TAPS):
            sy, sx = 1 - dy, 1 - dx
            if ki == 0:
                # split off the last column: it reads W00's right halo (from
                # W02), which would stall the whole mul on tap 2's exp
                wZ = Wt[ki][:, r + sy:r + sy + 1, 2:1 + W].broadcast_to((P, NCH, W - 1))
                nc.vector.tensor_tensor(pr[:, 4 + ki, :, 0:W - 1],
                                        Xin[:, :, r + sy + 1, 2:1 + W], wZ, op=OP.mult)
                wZl = Wt[ki][:, r + sy:r + sy + 1, 1 + W:2 + W].broadcast_to((P, NCH, 1))
                nc.vector.tensor_tensor(pr[:, 4 + ki, :, W - 1:W],
                                        Xin[:, :, r + sy + 1, 1 + W:2 + W], wZl, op=OP.mult)
                continue
            wZ = Wt[ki][:, r + sy:r + sy + 1, 1 + sx:1 + sx + W].broadcast_to((P, NCH, W))
            nc.vector.tensor_tensor(pr[:, 4 + ki], Xin[:, :, r + sy + 1, 1 + sx:1 + sx + W],
                                    wZ, op=OP.mult)
        dn = ppsum.tile([P, W], F32, name="t", tag="ps", bufs=8)
        nc.tensor.matmul(dn[:], gid[:], ones[:], start=True, stop=False)
        for ki, (dy, dx) in enumerate(TAPS):
            sy, sx = 1 - dy, 1 - dx
            nc.tensor.matmul(dn[:], idt[:], Wt[ki][:, r, 1:1 + W],
                             start=False, stop=False)
            nc.tensor.matmul(dn[:], idt[:], Wt[ki][:, r + sy, 1 + sx:1 + sx + W],
                             start=False, stop=(ki == 3))
        rec = psmall.tile([P, W], F32, name="t", tag="rec", bufs=3)
        nc.vector.reciprocal_approx_fast(out=rec[:], in_=dn[:])
        for c in range(NCH):
            t = ppsum.tile([P, W], F32, name="t", tag="ps", bufs=8)
            nc.tensor.matmul(t[:], gid[:], Xin[:, c, r + 1, 1:1 + W],
                             start=True, stop=False)
            for ki in range(4):
                nc.tensor.matmul(t[:], idt[:], pr[:, ki, c], start=False, stop=False)
                nc.tensor.matmul(t[:], idt[:], pr[:, 4 + ki, c],
                                 start=False, stop=(ki == 3))
            nc.vector.tensor_tensor(Xout[:, c, r + 1, 1:1 + W], t[:], rec[:], op=OP.mult)
    _halo_x(nc, Xout)


def _median(nc, pools, Xin, Xout, yview=None):
    """3x3 median: row-phase (lo/med/hi over 3-row windows, consuming Xin's
    halos) then col-phase on free-dim shifted views. If yview is given the
    result rows are DMAed there instead of written to Xout."""
    px, pw, pd, pprod, ppsum, psmall = pools
    mn = lambda o, a, b: nc.vector.tensor_tensor(o, a, b, op=OP.min)
    mx = lambda o, a, b: nc.vector.tensor_max(o, a, b)
    for r in range(RR):
        R0, R1, R2 = Xin[:, :, r], Xin[:, :, r + 1], Xin[:, :, r + 2]
        t1 = psmall.tile([P, NCH, WP], F16, name="t", tag="m1", bufs=2)
        t2 = psmall.tile([P, NCH, WP], F16, name="t", tag="m2", bufs=2)
        lo = psmall.tile([P, NCH, WP], F16, name="t", tag="m3", bufs=1)
        hi = psmall.tile([P, NCH, WP], F16, name="t", tag="m4", bufs=1)
        mn(t1[:], R0, R1)
        mx(t2[:], R0, R1)
        mn(lo[:], t1[:], R2)
        mx(hi[:], t2[:], R2)
        mn(t2[:], t2[:], R2)          # med partial
        mx(t1[:], t1[:], t2[:])       # t1 = med3 of rows
        m = t1
        AS, BS, CS = slice(0, W), slice(1, 1 + W), slice(2, 2 + W)
        H = psmall.tile([P, NCH, W], F16, name="t", tag="n1", bufs=1)
        L = psmall.tile([P, NCH, W], F16, name="t", tag="n2", bufs=1)
        u1 = psmall.tile([P, NCH, W], F16, name="t", tag="n3", bufs=1)
        u2 = psmall.tile([P, NCH, W], F16, name="t", tag="n4", bufs=1)
        v1 = psmall.tile([P, NCH, W], F16, name="t", tag="n4", bufs=1)
        mn(H[:], hi[:, :, AS], hi[:, :, BS])
        mn(H[:], H[:], hi[:, :, CS])
        mx(L[:], lo[:, :, AS], lo[:, :, BS])
        mx(L[:], L[:], lo[:, :, CS])
        mn(u1[:], m[:, :, AS], m[:, :, BS])
        mx(u2[:], m[:, :, AS], m[:, :, BS])
        mn(u2[:], u2[:], m[:, :, CS])
        mx(u1[:], u1[:], u2[:])       # u1 = M
        mn(v1[:], H[:], u1[:])
        mx(H[:], H[:], u1[:])
        mn(H[:], H[:], L[:])
        if yview is not None:
            orow = psmall.tile([P, NCH, W], F16, name="t", tag="or", bufs=1)
            mx(orow[:], v1[:], H[:])
            nc.sync.dma_start(out=yview[:, :, r:r + 1, :], in_=orow[:].unsqueeze(2))
        else:
            mx(Xout[:, :, r + 1, 1:1 + W], v1[:], H[:])
    if yview is None:
        _halo_x(nc, Xout)


def build():
    nc = bacc.Bacc("TRN2", target_bir_lowering=False, debug=False)
    _register_consts(nc)
    xin = nc.dram_tensor("xin", [NIMG, P, NCH, H6, WP], F16, kind="ExternalInput").ap()
    idg = nc.dram_tensor("idg", [2, P, P], F16, kind="ExternalInput").ap()
    yout = nc.dram_tensor("yout", [NIMG, P, NCH, RR, W], F16, kind="ExternalOutput").ap()

    with tile.TileContext(nc) as tc, ExitStack() as ctx:
        px = ctx.enter_context(tc.tile_pool(name="px", bufs=1))
        pw = ctx.enter_context(tc.tile_pool(name="pw", bufs=1))
        pd = ctx.enter_context(tc.tile_pool(name="pd", bufs=1))
        pprod = ctx.enter_context(tc.tile_pool(name="pprod", bufs=1))
        ppsum = ctx.enter_context(tc.psum_pool(name="ppsum", bufs=1))
        psmall = ctx.enter_context(tc.tile_pool(name="psmall", bufs=1))
        pools = (px, pw, pd, pprod, ppsum, psmall)

        idt = psmall.tile([P, P], F16, name="t", tag="id", bufs=1)
        nc.sync.dma_start(out=idt[:], in_=idg[0])
        gid = psmall.tile([P, P], F16, name="t", tag="gid", bufs=1)
        nc.sync.dma_start(out=gid[:], in_=idg[1])
        ones = psmall.tile([P, W], F16, name="t", tag="ones", bufs=1)
        nc.gpsimd.memset(ones[:], 1.0)

        X = []
        for img in range(NIMG):
            xt = px.tile([P, NCH, H6, WP], F16, name="t", tag="x", bufs=3)
            nc.gpsimd.dma_start(out=xt[:], in_=xin[img])
            X.append(xt)
        for img in range(NIMG):
            o = px.tile([P, NCH, H6, WP], F16, name="t", tag="x", bufs=3)
            _bilateral(nc, pools, X[img], o, SCALE1, idt, gid, ones)
            X[img] = o
        o = px.tile([P, NCH, H6, WP], F16, name="t", tag="x", bufs=3)
        _bilateral(nc, pools, X[0], o, SCALE2, idt, gid, ones)
        X[0] = o
        o = px.tile([P, NCH, H6, WP], F16, name="t", tag="x", bufs=3)
        _median(nc, pools, X[0], o)
        m0 = o
        o = px.tile([P, NCH, H6, WP], F16, name="t", tag="x", bufs=3)
        _bilateral(nc, pools, X[1], o, SCALE2, idt, gid, ones)
        X[1] = o
        _median(nc, pools, m0, None, yview=yout[0])
        o = px.tile([P, NCH, H6, WP], F16, name="t", tag="x", bufs=3)
        _median(nc, pools, X[1], o)
        X[1] = o
        _median(nc, pools, X[1], None, yview=yout[1])

    nc.compile()
    return nc


_NC_CACHE = None


def _get_nc():
    global _NC_CACHE
    if _NC_CACHE is None:
        _NC_CACHE = build()
    return _NC_CACHE


def _prep_inputs(x):
    xpad = np.pad(x, ((0, 0), (0, 0), (1, 1), (1, 1)), mode="reflect")
    rows = np.arange(P)[:, None] * RR + np.arange(H6)[None, :]
    win = xpad[:, :, rows, :]                          # (16,3,128,6,514)
    win = win.transpose(0, 2, 1, 3, 4).astype(np.float16)
    return np.ascontiguousarray(win.reshape(N_CORES, NIMG, P, NCH, H6, WP))


def kernel(x):
    x = np.ascontiguousarray(np.asarray(x), dtype=np.float32)
    assert x.shape == (16, 3, 512, 512)
    nc = _get_nc()
    xin = _prep_inputs(x)
    idg = np.ascontiguousarray(
        np.stack([np.eye(P), GC * np.eye(P)]).astype(np.float16))
    in_maps = [{"xin": xin[c], "idg": idg} for c in range(N_CORES)]
    res = run_bass_kernel_spmd(nc, in_maps, list(range(N_CORES)))
    out = np.empty((16, 3, 512, 512), np.float32)
    for c in range(N_CORES):
        y = res.results[c]["yout"]                     # (2,128,3,4,512) f16
        out[2 * c:2 * c + 2] = (y.transpose(0, 2, 1, 3, 4)
                                 .reshape(NIMG, NCH, 512, 512).astype(np.float32))
    return out


# revision 27
# speedup vs baseline: 1.0672x; 1.0250x over previous
"""Trainium2 Bass kernel for Bil_layer: 2x bilateral(3x3) + 2x median(3x3).

Data parallel: 2 images x 3 channels per core across 8 cores. Layout per
512x512 plane: 128 partitions x 4 rows; padded tile [128, 3ch, 6, 514]
holds rows -1..4 and cols -1..512 (reflect halos).

Bilateral uses the mirror-weight identity w_{2-dy,2-dx}(y,x) =
w_{dy,dx}(y+dy',x+dx') (dy'=1-dy, dx'=1-dx), so only 4 of 8 tap weights
are computed; mirrored taps read shifted views. Weighted sums accumulate
in PSUM fp32 via PE identity matmuls. Median runs row-phase first
(consuming the input tile's halos directly), then column-phase on free-dim
shifted views -- no intermediate halo exchange.
"""
import numpy as np
from contextlib import ExitStack

import concourse.tile as tile
from concourse import bacc, mybir
from concourse.bass_utils import run_bass_kernel_spmd

P = 128
RR = 4            # data rows per partition
H6 = 6            # padded rows (-1..4)
W = 512
WP = 514          # padded cols (-1..512)
NCH = 3
NIMG = 2
N_CORES = 8

SIGMA_COLOR = 0.1
COLOR2 = 0.01
SIGMA_SPACE = 10.0

F16 = mybir.dt.float16
F32 = mybir.dt.float32
OP = mybir.AluOpType
AF = mybir.ActivationFunctionType


def _gauss1():
    ax = np.arange(3, dtype=np.float64) - 1.0
    g = np.exp(-0.5 * (ax / SIGMA_SPACE) ** 2)
    return g / g.sum()


GO = _gauss1()
GC = float(GO[1] * GO[1])                 # center weight
LNG_C = float(np.log(GO[0] * GO[0]))      # ln g for taps (0,0),(0,2)
LNG_E = float(np.log(GO[0] * GO[1]))      # ln g for taps (0,1),(1,0)
TAPS = [(0, 0), (0, 1), (0, 2), (1, 0)]   # first-half taps; mirror shift = (1-dy, 1-dx)
LNG = [LNG_C, LNG_E, LNG_C, LNG_E]
SCALE1 = float(-0.5 / SIGMA_COLOR ** 2)
SCALE2 = float(-0.5 / COLOR2 ** 2)


def _register_consts(nc):
    for v in sorted({LNG_C, LNG_E}):
        if (F32, v) in nc.const_aps.aps:
            continue
        t = nc.alloc_sbuf_tensor(f"const-f32-{abs(hash(v))}", [P, 1], F32)
        nc.gpsimd.memset(t.ap(), v)
        nc.const_aps.aps[(F32, v)] = t.ap()
    nc.all_engine_barrier()


def _halo_x(nc, X):
    """Fill halos of padded tile X [P, NCH, 6, 514] whose interior
    (rows 1..4, cols 1..512) was written. Col reflect copies first, then
    full-width row halos (so corners ride along)."""
    nc.gpsimd.tensor_copy(out=X[:, :, 1:5, 0:1], in_=X[:, :, 1:5, 2:3])
    nc.gpsimd.tensor_copy(out=X[:, :, 1:5, WP - 1:WP], in_=X[:, :, 1:5, WP - 3:WP - 2])
    nc.gpsimd.dma_start(out=X[0:1, :, 0:1, :], in_=X[0:1, :, 2:3, :])
    nc.gpsimd.dma_start(out=X[P - 1:P, :, 5:6, :], in_=X[P - 1:P, :, 3:4, :])
    nc.sync.dma_start(out=X[1:P, :, 0:1, :], in_=X[0:P - 1, :, 4:5, :])
    nc.sync.dma_start(out=X[0:P - 1, :, 5:6, :], in_=X[1:P, :, 1:2, :])


def _bilateral(nc, pools, Xin, Xout, scale, idt, gid, ones):
    px, pw, pd, pprod, ppsum, psmall = pools

    # --- per-tap color weights (4 first-half taps) ---
    Wt = [None] * 4
    for emi, ki in enumerate((0, 2, 1, 3)):
        dy, dx = TAPS[ki]
        d = pd.tile([P, NCH, RR, W], F16, name="t", tag="d", bufs=4)
        eng = nc.vector if emi < 2 else nc.gpsimd
        eng.tensor_tensor(d[:], Xin[:, :, dy:dy + RR, dx:dx + W],
                          Xin[:, :, 1:5, 1:1 + W], op=OP.subtract)
        nc.scalar.activation(out=d[:], in_=d[:], func=AF.Abs)
        eng.tensor_tensor(d[:, 0], d[:, 0], d[:, 1], op=OP.add)
        eng.tensor_tensor(d[:, 0], d[:, 0], d[:, 2], op=OP.add)
        nc.scalar.activation(out=d[:, 0], in_=d[:, 0], func=AF.Square)
        wk = pw.tile([P, RR + 1, WP], F16, name="t", tag=f"w{ki}", bufs=1)
        nc.scalar.activation(out=wk[:, 0:RR, 1:1 + W], in_=d[:, 0], func=AF.Exp,
                             bias=LNG[ki], scale=scale)
        Wt[ki] = wk

    # --- W halo fixups (mirror taps read shifted views) ---
    # col halos: W00 right <- W02 col x=510; W02 left <- W00 col x=1; W10 right <- own x=511
    nc.gpsimd.tensor_copy(out=Wt[0][:, 0:RR, WP - 1:WP], in_=Wt[2][:, 0:RR, W - 1:W])
    nc.gpsimd.tensor_copy(out=Wt[2][:, 0:RR, 0:1], in_=Wt[0][:, 0:RR, 2:3])
    nc.gpsimd.tensor_copy(out=Wt[3][:, 0:RR, WP - 1:WP], in_=Wt[3][:, 0:RR, WP - 2:WP - 1])
    # bottom halo rows (taps with mirror dy-shift): interior partitions
    for k in (0, 1, 2):
        nc.sync.dma_start(out=Wt[k][0:P - 1, RR:RR + 1, :], in_=Wt[k][1:P, 0:1, :])
    # last partition bottom rows via the reflect partner plane
    nc.gpsimd.dma_start(out=Wt[0][P - 1:P, RR:RR + 1, 2:WP],
                        in_=Wt[2][P - 1:P, RR - 1:RR, 1:WP - 1])
    nc.gpsimd.dma_start(out=Wt[1][P - 1:P, RR:RR + 1, 1:WP - 1],
                        in_=Wt[1][P - 1:P, RR - 1:RR, 1:WP - 1])
    nc.gpsimd.dma_start(out=Wt[2][P - 1:P, RR:RR + 1, 0:W],
                        in_=Wt[0][P - 1:P, RR - 1:RR, 1:WP - 1])

    # --- per-row products + PE accumulation + normalize ---
    for r in range(RR):
        pr = pprod.tile([P, 8, NCH, W], F16, name="t", tag="prod", bufs=1)
        for ki, (dy, dx) in enumerate(TAPS):
            wQ = Wt[ki][:, r:r + 1, 1:1 + W].broadcast_to((P, NCH, W))
            nc.vector.tensor_tensor(pr[:, ki], Xin[:, :, dy + r, dx:dx + W], wQ,
                                    op=OP.mult)
        for ki, (dy, dx) in enumerate(TAPS):
            sy, sx = 1 - dy, 1 - dx
            if ki == 0:
                # split off the last column: it reads W00's right halo (from
                # W02), which would stall the whole mul on tap 2's exp
                wZ = Wt[ki][:, r + sy:r + sy + 1, 2:1 + W].broadcast_to((P, NCH, W - 1))
                nc.vector.tensor_tensor(pr[:, 4 + ki, :, 0:W - 1],
                                        Xin[:, :, r + sy + 1, 2:1 + W], wZ, op=OP.mult)
                wZl = Wt[ki][:, r + sy:r + sy + 1, 1 + W:2 + W].broadcast_to((P, NCH, 1))
                nc.vector.tensor_tensor(pr[:, 4 + ki, :, W - 1:W],
                                        Xin[:, :, r + sy + 1, 1 + W:2 + W], wZl, op=OP.mult)
                continue
            wZ = Wt[ki][:, r + sy:r + sy + 1, 1 + sx:1 + sx + W].broadcast_to((P, NCH, W))
            nc.vector.tensor_tensor(pr[:, 4 + ki], Xin[:, :, r + sy + 1, 1 + sx:1 + sx + W],
                                    wZ, op=OP.mult)
        dn = ppsum.tile([P, W], F32, name="t", tag="ps", bufs=6)
        nc.tensor.matmul(dn[:], gid[:], ones[:], start=True, stop=False)
        for ki, (dy, dx) in enumerate(TAPS):
            sy, sx = 1 - dy, 1 - dx
            nc.tensor.matmul(dn[:], idt[:], Wt[ki][:, r, 1:1 + W],
                             start=False, stop=False)
            nc.tensor.matmul(dn[:], idt[:], Wt[ki][:, r + sy, 1 + sx:1 + sx + W],
                             start=False, stop=(ki == 3))
        rec = psmall.tile([P, W], F32, name="t", tag="rec", bufs=3)
        nc.vector.reciprocal_approx_fast(out=rec[:], in_=dn[:])
        for c in range(NCH):
            t = ppsum.tile([P, W], F32, name="t", tag="ps", bufs=6)
            nc.tensor.matmul(t[:], gid[:], Xin[:, c, r + 1, 1:1 + W],
                             start=True, stop=False)
            for ki in range(4):
                nc.tensor.matmul(t[:], idt[:], pr[:, ki, c], start=False, stop=False)
                nc.tensor.matmul(t[:], idt[:], pr[:, 4 + ki, c],
                                 start=False, stop=(ki == 3))
            nc.vector.tensor_tensor(Xout[:, c, r + 1, 1:1 + W], t[:], rec[:], op=OP.mult)
    _halo_x(nc, Xout)


def _median(nc, pools, Xin, Xout, yview=None):
    """3x3 median: row-phase (lo/med/hi over 3-row windows, consuming Xin's
    halos) then col-phase on free-dim shifted views. If yview is given the
    result rows are DMAed there instead of written to Xout."""
    px, pw, pd, pprod, ppsum, psmall = pools
    mn = lambda o, a, b: nc.vector.tensor_tensor(o, a, b, op=OP.min)
    mx = lambda o, a, b: nc.vector.tensor_max(o, a, b)
    for r in range(RR):
        R0, R1, R2 = Xin[:, :, r], Xin[:, :, r + 1], Xin[:, :, r + 2]
        t1 = psmall.tile([P, NCH, WP], F16, name="t", tag="m1", bufs=2)
        t2 = psmall.tile([P, NCH, WP], F16, name="t", tag="m2", bufs=2)
        lo = psmall.tile([P, NCH, WP], F16, name="t", tag="m3", bufs=1)
        hi = psmall.tile([P, NCH, WP], F16, name="t", tag="m4", bufs=1)
        mn(t1[:], R0, R1)
        mx(t2[:], R0, R1)
        mn(lo[:], t1[:], R2)
        mx(hi[:], t2[:], R2)
        mn(t2[:], t2[:], R2)          # med partial
        mx(t1[:], t1[:], t2[:])       # t1 = med3 of rows
        m = t1
        AS, BS, CS = slice(0, W), slice(1, 1 + W), slice(2, 2 + W)
        H = psmall.tile([P, NCH, W], F16, name="t", tag="n1", bufs=1)
        L = psmall.tile([P, NCH, W], F16, name="t", tag="n2", bufs=1)
        u1 = psmall.tile([P, NCH, W], F16, name="t", tag="n3", bufs=1)
        u2 = psmall.tile([P, NCH, W], F16, name="t", tag="n4", bufs=1)
        v1 = psmall.tile([P, NCH, W], F16, name="t", tag="n4", bufs=1)
        mn(H[:], hi[:, :, AS], hi[:, :, BS])
        mn(H[:], H[:], hi[:, :, CS])
        mx(L[:], lo[:, :, AS], lo[:, :, BS])
        mx(L[:], L[:], lo[:, :, CS])
        mn(u1[:], m[:, :, AS], m[:, :, BS])
        mx(u2[:], m[:, :, AS], m[:, :, BS])
        mn(u2[:], u2[:], m[:, :, CS])
        mx(u1[:], u1[:], u2[:])       # u1 = M
        mn(v1[:], H[:], u1[:])
        mx(H[:], H[:], u1[:])
        mn(H[:], H[:], L[:])
        if yview is not None:
            orow = psmall.tile([P, NCH, W], F16, name="t", tag="or", bufs=1)
            mx(orow[:], v1[:], H[:])
            nc.sync.dma_start(out=yview[:, :, r:r + 1, :], in_=orow[:].unsqueeze(2))
        else:
            mx(Xout[:, :, r + 1, 1:1 + W], v1[:], H[:])
    if yview is None:
        _halo_x(nc, Xout)


def build():
    nc = bacc.Bacc("TRN2", target_bir_lowering=False, debug=False)
    _register_consts(nc)
    xin = nc.dram_tensor("xin", [NIMG, P, NCH, H6, WP], F16, kind="ExternalInput").ap()
    idg = nc.dram_tensor("idg", [2, P, P], F16, kind="ExternalInput").ap()
    yout = nc.dram_tensor("yout", [NIMG, P, NCH, RR, W], F16, kind="ExternalOutput").ap()

    with tile.TileContext(nc) as tc, ExitStack() as ctx:
        px = ctx.enter_context(tc.tile_pool(name="px", bufs=1))
        pw = ctx.enter_context(tc.tile_pool(name="pw", bufs=1))
        pd = ctx.enter_context(tc.tile_pool(name="pd", bufs=1))
        pprod = ctx.enter_context(tc.tile_pool(name="pprod", bufs=1))
        ppsum = ctx.enter_context(tc.psum_pool(name="ppsum", bufs=1))
        psmall = ctx.enter_context(tc.tile_pool(name="psmall", bufs=1))
        pools = (px, pw, pd, pprod, ppsum, psmall)

        idt = psmall.tile([P, P], F16, name="t", tag="id", bufs=1)
        nc.sync.dma_start(out=idt[:], in_=idg[0])
        gid = psmall.tile([P, P], F16, name="t", tag="gid", bufs=1)
        nc.sync.dma_start(out=gid[:], in_=idg[1])
        ones = psmall.tile([P, W], F16, name="t", tag="ones", bufs=1)
        nc.gpsimd.memset(ones[:], 1.0)

        X = []
        for img in range(NIMG):
            xt = px.tile([P, NCH, H6, WP], F16, name="t", tag="x", bufs=4)
            nc.gpsimd.dma_start(out=xt[:], in_=xin[img])
            X.append(xt)
        for img in range(NIMG):
            o = px.tile([P, NCH, H6, WP], F16, name="t", tag="x", bufs=4)
            _bilateral(nc, pools, X[img], o, SCALE1, idt, gid, ones)
            X[img] = o
        o = px.tile([P, NCH, H6, WP], F16, name="t", tag="x", bufs=4)
        _bilateral(nc, pools, X[0], o, SCALE2, idt, gid, ones)
        X[0] = o
        o = px.tile([P, NCH, H6, WP], F16, name="t", tag="x", bufs=4)
        _median(nc, pools, X[0], o)
        m0 = o
        o = px.tile([P, NCH, H6, WP], F16, name="t", tag="x", bufs=4)
        _bilateral(nc, pools, X[1], o, SCALE2, idt, gid, ones)
        X[1] = o
        _median(nc, pools, m0, None, yview=yout[0])
        o = px.tile([P, NCH, H6, WP], F16, name="t", tag="x", bufs=4)
        _median(nc, pools, X[1], o)
        X[1] = o
        _median(nc, pools, X[1], None, yview=yout[1])

    nc.compile()
    return nc


_NC_CACHE = None


def _get_nc():
    global _NC_CACHE
    if _NC_CACHE is None:
        _NC_CACHE = build()
    return _NC_CACHE


def _prep_inputs(x):
    xpad = np.pad(x, ((0, 0), (0, 0), (1, 1), (1, 1)), mode="reflect")
    rows = np.arange(P)[:, None] * RR + np.arange(H6)[None, :]
    win = xpad[:, :, rows, :]                          # (16,3,128,6,514)
    win = win.transpose(0, 2, 1, 3, 4).astype(np.float16)
    return np.ascontiguousarray(win.reshape(N_CORES, NIMG, P, NCH, H6, WP))


def kernel(x):
    x = np.ascontiguousarray(np.asarray(x), dtype=np.float32)
    assert x.shape == (16, 3, 512, 512)
    nc = _get_nc()
    xin = _prep_inputs(x)
    idg = np.ascontiguousarray(
        np.stack([np.eye(P), GC * np.eye(P)]).astype(np.float16))
    in_maps = [{"xin": xin[c], "idg": idg} for c in range(N_CORES)]
    res = run_bass_kernel_spmd(nc, in_maps, list(range(N_CORES)))
    out = np.empty((16, 3, 512, 512), np.float32)
    for c in range(N_CORES):
        y = res.results[c]["yout"]                     # (2,128,3,4,512) f16
        out[2 * c:2 * c + 2] = (y.transpose(0, 2, 1, 3, 4)
                                 .reshape(NIMG, NCH, 512, 512).astype(np.float32))
    return out


# revision 31
# speedup vs baseline: 1.0759x; 1.0081x over previous
"""Trainium2 Bass kernel for Bil_layer: 2x bilateral(3x3) + 2x median(3x3).

Data parallel: 2 images x 3 channels per core across 8 cores. Layout per
512x512 plane: 128 partitions x 4 rows; padded tile [128, 3ch, 6, 514]
holds rows -1..4 and cols -1..512 (reflect halos).

Bilateral uses the mirror-weight identity w_{2-dy,2-dx}(y,x) =
w_{dy,dx}(y+dy',x+dx') (dy'=1-dy, dx'=1-dx), so only 4 of 8 tap weights
are computed; mirrored taps read shifted views. Weighted sums accumulate
in PSUM fp32 via PE identity matmuls. Median runs row-phase first
(consuming the input tile's halos directly), then column-phase on free-dim
shifted views -- no intermediate halo exchange.
"""
import numpy as np
from contextlib import ExitStack

import concourse.tile as tile
from concourse import bacc, mybir
from concourse.bass_utils import run_bass_kernel_spmd

P = 128
RR = 4            # data rows per partition
H6 = 6            # padded rows (-1..4)
W = 512
WP = 514          # padded cols (-1..512)
NCH = 3
NIMG = 2
N_CORES = 8

SIGMA_COLOR = 0.1
COLOR2 = 0.01
SIGMA_SPACE = 10.0

F16 = mybir.dt.float16
F32 = mybir.dt.float32
OP = mybir.AluOpType
AF = mybir.ActivationFunctionType


def _gauss1():
    ax = np.arange(3, dtype=np.float64) - 1.0
    g = np.exp(-0.5 * (ax / SIGMA_SPACE) ** 2)
    return g / g.sum()


GO = _gauss1()
GC = float(GO[1] * GO[1])                 # center weight
LNG_C = float(np.log(GO[0] * GO[0]))      # ln g for taps (0,0),(0,2)
LNG_E = float(np.log(GO[0] * GO[1]))      # ln g for taps (0,1),(1,0)
TAPS = [(0, 0), (0, 1), (0, 2), (1, 0)]   # first-half taps; mirror shift = (1-dy, 1-dx)
LNG = [LNG_C, LNG_E, LNG_C, LNG_E]
SCALE1 = float(-0.5 / SIGMA_COLOR ** 2)
SCALE2 = float(-0.5 / COLOR2 ** 2)


def _register_consts(nc):
    for v in sorted({LNG_C, LNG_E}):
        if (F32, v) in nc.const_aps.aps:
            continue
        t = nc.alloc_sbuf_tensor(f"const-f32-{abs(hash(v))}", [P, 1], F32)
        nc.gpsimd.memset(t.ap(), v)
        nc.const_aps.aps[(F32, v)] = t.ap()
    nc.all_engine_barrier()


def _halo_x(nc, X):
    """Fill halos of padded tile X [P, NCH, 6, 514] whose interior
    (rows 1..4, cols 1..512) was written. Col reflect copies first, then
    full-width row halos (so corners ride along)."""
    nc.gpsimd.tensor_copy(out=X[:, :, 1:5, 0:1], in_=X[:, :, 1:5, 2:3])
    nc.gpsimd.tensor_copy(out=X[:, :, 1:5, WP - 1:WP], in_=X[:, :, 1:5, WP - 3:WP - 2])
    nc.gpsimd.dma_start(out=X[0:1, :, 0:1, :], in_=X[0:1, :, 2:3, :])
    nc.gpsimd.dma_start(out=X[P - 1:P, :, 5:6, :], in_=X[P - 1:P, :, 3:4, :])
    nc.sync.dma_start(out=X[1:P, :, 0:1, :], in_=X[0:P - 1, :, 4:5, :])
    nc.sync.dma_start(out=X[0:P - 1, :, 5:6, :], in_=X[1:P, :, 1:2, :])


def _bilateral(nc, pools, Xin, Xout, scale, idt, gid, ones):
    px, pw, pd, pprod, ppsum, psmall = pools

    # --- per-tap color weights (4 first-half taps) ---
    Wt = [None] * 4
    for emi, ki in enumerate((0, 1, 2, 3)):
        dy, dx = TAPS[ki]
        d = pd.tile([P, NCH, RR, W], F16, name="t", tag=f"d{emi % 2}", bufs=2)
        eng = nc.vector if emi < 2 else nc.gpsimd
        eng.tensor_tensor(d[:], Xin[:, :, dy:dy + RR, dx:dx + W],
                          Xin[:, :, 1:5, 1:1 + W], op=OP.subtract)
        nc.scalar.activation(out=d[:], in_=d[:], func=AF.Abs)
        eng.tensor_tensor(d[:, 0], d[:, 0], d[:, 1], op=OP.add)
        eng.tensor_tensor(d[:, 0], d[:, 0], d[:, 2], op=OP.add)
        nc.scalar.activation(out=d[:, 0], in_=d[:, 0], func=AF.Square)
        wk = pw.tile([P, RR + 1, WP], F16, name="t", tag=f"w{ki}", bufs=1)
        nc.scalar.activation(out=wk[:, 0:RR, 1:1 + W], in_=d[:, 0], func=AF.Exp,
                             bias=LNG[ki], scale=scale)
        Wt[ki] = wk

    # --- W halo fixups (mirror taps read shifted views) ---
    # col halos: W00 right <- W02 col x=510; W02 left <- W00 col x=1; W10 right <- own x=511
    nc.gpsimd.tensor_copy(out=Wt[0][:, 0:RR, WP - 1:WP], in_=Wt[2][:, 0:RR, W - 1:W])
    nc.gpsimd.tensor_copy(out=Wt[2][:, 0:RR, 0:1], in_=Wt[0][:, 0:RR, 2:3])
    nc.gpsimd.tensor_copy(out=Wt[3][:, 0:RR, WP - 1:WP], in_=Wt[3][:, 0:RR, WP - 2:WP - 1])
    # bottom halo rows (taps with mirror dy-shift): interior partitions
    for k in (0, 1, 2):
        nc.sync.dma_start(out=Wt[k][0:P - 1, RR:RR + 1, :], in_=Wt[k][1:P, 0:1, :])
    # last partition bottom rows via the reflect partner plane
    nc.gpsimd.dma_start(out=Wt[0][P - 1:P, RR:RR + 1, 2:WP],
                        in_=Wt[2][P - 1:P, RR - 1:RR, 1:WP - 1])
    nc.gpsimd.dma_start(out=Wt[1][P - 1:P, RR:RR + 1, 1:WP - 1],
                        in_=Wt[1][P - 1:P, RR - 1:RR, 1:WP - 1])
    nc.gpsimd.dma_start(out=Wt[2][P - 1:P, RR:RR + 1, 0:W],
                        in_=Wt[0][P - 1:P, RR - 1:RR, 1:WP - 1])

    # --- per-row products + PE accumulation + normalize ---
    for r in range(RR):
        pr = pprod.tile([P, 8, NCH, W], F16, name="t", tag="prod", bufs=1)
        for ki, (dy, dx) in enumerate(TAPS):
            wQ = Wt[ki][:, r:r + 1, 1:1 + W].broadcast_to((P, NCH, W))
            nc.vector.tensor_tensor(pr[:, ki], Xin[:, :, dy + r, dx:dx + W], wQ,
                                    op=OP.mult)
        for ki, (dy, dx) in enumerate(TAPS):
            sy, sx = 1 - dy, 1 - dx
            if ki == 0:
                # split off the last column: it reads W00's right halo (from
                # W02), which would stall the whole mul on tap 2's exp
                wZ = Wt[ki][:, r + sy:r + sy + 1, 2:1 + W].broadcast_to((P, NCH, W - 1))
                nc.vector.tensor_tensor(pr[:, 4 + ki, :, 0:W - 1],
                                        Xin[:, :, r + sy + 1, 2:1 + W], wZ, op=OP.mult)
                wZl = Wt[ki][:, r + sy:r + sy + 1, 1 + W:2 + W].broadcast_to((P, NCH, 1))
                nc.vector.tensor_tensor(pr[:, 4 + ki, :, W - 1:W],
                                        Xin[:, :, r + sy + 1, 1 + W:2 + W], wZl, op=OP.mult)
                continue
            wZ = Wt[ki][:, r + sy:r + sy + 1, 1 + sx:1 + sx + W].broadcast_to((P, NCH, W))
            nc.vector.tensor_tensor(pr[:, 4 + ki], Xin[:, :, r + sy + 1, 1 + sx:1 + sx + W],
                                    wZ, op=OP.mult)
        dn = ppsum.tile([P, W], F32, name="t", tag="psd", bufs=3)
        nc.tensor.matmul(dn[:], gid[:], ones[:], start=True, stop=False)
        for ki, (dy, dx) in enumerate(TAPS):
            sy, sx = 1 - dy, 1 - dx
            nc.tensor.matmul(dn[:], idt[:], Wt[ki][:, r, 1:1 + W],
                             start=False, stop=False)
            nc.tensor.matmul(dn[:], idt[:], Wt[ki][:, r + sy, 1 + sx:1 + sx + W],
                             start=False, stop=(ki == 3))
        rec = psmall.tile([P, W], F32, name="t", tag="rec", bufs=3)
        nc.vector.reciprocal_approx_fast(out=rec[:], in_=dn[:])
        for c in range(NCH):
            t = ppsum.tile([P, W], F32, name="t", tag="pst", bufs=5)
            nc.tensor.matmul(t[:], gid[:], Xin[:, c, r + 1, 1:1 + W],
                             start=True, stop=False)
            for ki in range(4):
                nc.tensor.matmul(t[:], idt[:], pr[:, ki, c], start=False, stop=False)
                nc.tensor.matmul(t[:], idt[:], pr[:, 4 + ki, c],
                                 start=False, stop=(ki == 3))
            nc.vector.tensor_tensor(Xout[:, c, r + 1, 1:1 + W], t[:], rec[:], op=OP.mult)
    _halo_x(nc, Xout)


def _median(nc, pools, Xin, Xout, yview=None):
    """3x3 median: row-phase (lo/med/hi over 3-row windows, consuming Xin's
    halos) then col-phase on free-dim shifted views. If yview is given the
    result rows are DMAed there instead of written to Xout."""
    px, pw, pd, pprod, ppsum, psmall = pools
    mn = lambda o, a, b: nc.vector.tensor_tensor(o, a, b, op=OP.min)
    mx = lambda o, a, b: nc.vector.tensor_max(o, a, b)
    for r in range(RR):
        R0, R1, R2 = Xin[:, :, r], Xin[:, :, r + 1], Xin[:, :, r + 2]
        t1 = psmall.tile([P, NCH, WP], F16, name="t", tag="m1", bufs=2)
        t2 = psmall.tile([P, NCH, WP], F16, name="t", tag="m2", bufs=2)
        lo = psmall.tile([P, NCH, WP], F16, name="t", tag="m3", bufs=1)
        hi = psmall.tile([P, NCH, WP], F16, name="t", tag="m4", bufs=1)
        mn(t1[:], R0, R1)
        mx(t2[:], R0, R1)
        mn(lo[:], t1[:], R2)
        mx(hi[:], t2[:], R2)
        mn(t2[:], t2[:], R2)          # med partial
        mx(t1[:], t1[:], t2[:])       # t1 = med3 of rows
        m = t1
        AS, BS, CS = slice(0, W), slice(1, 1 + W), slice(2, 2 + W)
        H = psmall.tile([P, NCH, W], F16, name="t", tag="n1", bufs=1)
        L = psmall.tile([P, NCH, W], F16, name="t", tag="n2", bufs=1)
        u1 = psmall.tile([P, NCH, W], F16, name="t", tag="n3", bufs=1)
        u2 = psmall.tile([P, NCH, W], F16, name="t", tag="n4", bufs=1)
        v1 = psmall.tile([P, NCH, W], F16, name="t", tag="n4", bufs=1)
        mn(H[:], hi[:, :, AS], hi[:, :, BS])
        mn(H[:], H[:], hi[:, :, CS])
        mx(L[:], lo[:, :, AS], lo[:, :, BS])
        mx(L[:], L[:], lo[:, :, CS])
        mn(u1[:], m[:, :, AS], m[:, :, BS])
        mx(u2[:], m[:, :, AS], m[:, :, BS])
        mn(u2[:], u2[:], m[:, :, CS])
        mx(u1[:], u1[:], u2[:])       # u1 = M
        mn(v1[:], H[:], u1[:])
        mx(H[:], H[:], u1[:])
        mn(H[:], H[:], L[:])
        if yview is not None:
            orow = psmall.tile([P, NCH, W], F16, name="t", tag="or", bufs=1)
            mx(orow[:], v1[:], H[:])
            nc.sync.dma_start(out=yview[:, :, r:r + 1, :], in_=orow[:].unsqueeze(2))
        else:
            mx(Xout[:, :, r + 1, 1:1 + W], v1[:], H[:])
    if yview is None:
        _halo_x(nc, Xout)


def build():
    nc = bacc.Bacc("TRN2", target_bir_lowering=False, debug=False)
    _register_consts(nc)
    xin = nc.dram_tensor("xin", [NIMG, P, NCH, H6, WP], F16, kind="ExternalInput").ap()
    idg = nc.dram_tensor("idg", [2, P, P], F16, kind="ExternalInput").ap()
    yout = nc.dram_tensor("yout", [NIMG, P, NCH, RR, W], F16, kind="ExternalOutput").ap()

    with tile.TileContext(nc) as tc, ExitStack() as ctx:
        px = ctx.enter_context(tc.tile_pool(name="px", bufs=1))
        pw = ctx.enter_context(tc.tile_pool(name="pw", bufs=1))
        pd = ctx.enter_context(tc.tile_pool(name="pd", bufs=1))
        pprod = ctx.enter_context(tc.tile_pool(name="pprod", bufs=1))
        ppsum = ctx.enter_context(tc.psum_pool(name="ppsum", bufs=1))
        psmall = ctx.enter_context(tc.tile_pool(name="psmall", bufs=1))
        pools = (px, pw, pd, pprod, ppsum, psmall)

        idt = psmall.tile([P, P], F16, name="t", tag="id", bufs=1)
        nc.sync.dma_start(out=idt[:], in_=idg[0])
        gid = psmall.tile([P, P], F16, name="t", tag="gid", bufs=1)
        nc.sync.dma_start(out=gid[:], in_=idg[1])
        ones = psmall.tile([P, W], F16, name="t", tag="ones", bufs=1)
        nc.gpsimd.memset(ones[:], 1.0)

        X = []
        for img in range(NIMG):
            xt = px.tile([P, NCH, H6, WP], F16, name="t", tag="x", bufs=4)
            nc.gpsimd.dma_start(out=xt[:], in_=xin[img])
            X.append(xt)
        for img in range(NIMG):
            o = px.tile([P, NCH, H6, WP], F16, name="t", tag="x", bufs=4)
            _bilateral(nc, pools, X[img], o, SCALE1, idt, gid, ones)
            X[img] = o
        o = px.tile([P, NCH, H6, WP], F16, name="t", tag="x", bufs=4)
        _bilateral(nc, pools, X[0], o, SCALE2, idt, gid, ones)
        X[0] = o
        o = px.tile([P, NCH, H6, WP], F16, name="t", tag="x", bufs=4)
        _median(nc, pools, X[0], o)
        m0 = o
        o = px.tile([P, NCH, H6, WP], F16, name="t", tag="x", bufs=4)
        _bilateral(nc, pools, X[1], o, SCALE2, idt, gid, ones)
        X[1] = o
        _median(nc, pools, m0, None, yview=yout[0])
        o = px.tile([P, NCH, H6, WP], F16, name="t", tag="x", bufs=4)
        _median(nc, pools, X[1], o)
        X[1] = o
        _median(nc, pools, X[1], None, yview=yout[1])

    nc.compile()
    return nc


_NC_CACHE = None


def _get_nc():
    global _NC_CACHE
    if _NC_CACHE is None:
        _NC_CACHE = build()
    return _NC_CACHE


def _prep_inputs(x):
    xpad = np.pad(x, ((0, 0), (0, 0), (1, 1), (1, 1)), mode="reflect")
    rows = np.arange(P)[:, None] * RR + np.arange(H6)[None, :]
    win = xpad[:, :, rows, :]                          # (16,3,128,6,514)
    win = win.transpose(0, 2, 1, 3, 4).astype(np.float16)
    return np.ascontiguousarray(win.reshape(N_CORES, NIMG, P, NCH, H6, WP))


def kernel(x):
    x = np.ascontiguousarray(np.asarray(x), dtype=np.float32)
    assert x.shape == (16, 3, 512, 512)
    nc = _get_nc()
    xin = _prep_inputs(x)
    idg = np.ascontiguousarray(
        np.stack([np.eye(P), GC * np.eye(P)]).astype(np.float16))
    in_maps = [{"xin": xin[c], "idg": idg} for c in range(N_CORES)]
    res = run_bass_kernel_spmd(nc, in_maps, list(range(N_CORES)))
    out = np.empty((16, 3, 512, 512), np.float32)
    for c in range(N_CORES):
        y = res.results[c]["yout"]                     # (2,128,3,4,512) f16
        out[2 * c:2 * c + 2] = (y.transpose(0, 2, 1, 3, 4)
                                 .reshape(NIMG, NCH, 512, 512).astype(np.float32))
    return out


# revision 32
# speedup vs baseline: 1.0783x; 1.0022x over previous
"""Trainium2 Bass kernel for Bil_layer: 2x bilateral(3x3) + 2x median(3x3).

Data parallel: 2 images x 3 channels per core across 8 cores. Layout per
512x512 plane: 128 partitions x 4 rows; padded tile [128, 3ch, 6, 514]
holds rows -1..4 and cols -1..512 (reflect halos).

Bilateral uses the mirror-weight identity w_{2-dy,2-dx}(y,x) =
w_{dy,dx}(y+dy',x+dx') (dy'=1-dy, dx'=1-dx), so only 4 of 8 tap weights
are computed; mirrored taps read shifted views. Weighted sums accumulate
in PSUM fp32 via PE identity matmuls. Median runs row-phase first
(consuming the input tile's halos directly), then column-phase on free-dim
shifted views -- no intermediate halo exchange.
"""
import numpy as np
from contextlib import ExitStack

import concourse.tile as tile
from concourse import bacc, mybir
from concourse.bass_utils import run_bass_kernel_spmd

P = 128
RR = 4            # data rows per partition
H6 = 6            # padded rows (-1..4)
W = 512
WP = 514          # padded cols (-1..512)
NCH = 3
NIMG = 2
N_CORES = 8

SIGMA_COLOR = 0.1
COLOR2 = 0.01
SIGMA_SPACE = 10.0

F16 = mybir.dt.float16
F32 = mybir.dt.float32
OP = mybir.AluOpType
AF = mybir.ActivationFunctionType


def _gauss1():
    ax = np.arange(3, dtype=np.float64) - 1.0
    g = np.exp(-0.5 * (ax / SIGMA_SPACE) ** 2)
    return g / g.sum()


GO = _gauss1()
GC = float(GO[1] * GO[1])                 # center weight
LNG_C = float(np.log(GO[0] * GO[0]))      # ln g for taps (0,0),(0,2)
LNG_E = float(np.log(GO[0] * GO[1]))      # ln g for taps (0,1),(1,0)
TAPS = [(0, 0), (0, 1), (0, 2), (1, 0)]   # first-half taps; mirror shift = (1-dy, 1-dx)
LNG = [LNG_C, LNG_E, LNG_C, LNG_E]
SCALE1 = float(-0.5 / SIGMA_COLOR ** 2)
SCALE2 = float(-0.5 / COLOR2 ** 2)


def _register_consts(nc):
    for v in sorted({LNG_C, LNG_E}):
        if (F32, v) in nc.const_aps.aps:
            continue
        t = nc.alloc_sbuf_tensor(f"const-f32-{abs(hash(v))}", [P, 1], F32)
        nc.gpsimd.memset(t.ap(), v)
        nc.const_aps.aps[(F32, v)] = t.ap()
    nc.all_engine_barrier()


def _halo_x(nc, X):
    """Fill halos of padded tile X [P, NCH, 6, 514] whose interior
    (rows 1..4, cols 1..512) was written. Col reflect copies first, then
    full-width row halos (so corners ride along)."""
    nc.gpsimd.tensor_copy(out=X[:, :, 1:5, 0:1], in_=X[:, :, 1:5, 2:3])
    nc.gpsimd.tensor_copy(out=X[:, :, 1:5, WP - 1:WP], in_=X[:, :, 1:5, WP - 3:WP - 2])
    nc.gpsimd.dma_start(out=X[0:1, :, 0:1, :], in_=X[0:1, :, 2:3, :])
    nc.gpsimd.dma_start(out=X[P - 1:P, :, 5:6, :], in_=X[P - 1:P, :, 3:4, :])
    nc.sync.dma_start(out=X[1:P, :, 0:1, :], in_=X[0:P - 1, :, 4:5, :])
    nc.sync.dma_start(out=X[0:P - 1, :, 5:6, :], in_=X[1:P, :, 1:2, :])


def _bilateral(nc, pools, Xin, Xout, scale, idt, gid, ones):
    px, pw, pd, pprod, ppsum, psmall = pools

    # --- per-tap color weights (4 first-half taps) ---
    Wt = [None] * 4
    for emi, ki in enumerate((0, 1, 2, 3)):
        dy, dx = TAPS[ki]
        d = pd.tile([P, NCH, RR, W], F16, name="t", tag=f"d{emi % 2}", bufs=2)
        eng = nc.vector if emi < 2 else nc.gpsimd
        eng.tensor_tensor(d[:], Xin[:, :, dy:dy + RR, dx:dx + W],
                          Xin[:, :, 1:5, 1:1 + W], op=OP.subtract)
        nc.scalar.activation(out=d[:], in_=d[:], func=AF.Abs)
        eng.tensor_tensor(d[:, 0], d[:, 0], d[:, 1], op=OP.add)
        eng.tensor_tensor(d[:, 0], d[:, 0], d[:, 2], op=OP.add)
        nc.scalar.activation(out=d[:, 0], in_=d[:, 0], func=AF.Square)
        wk = pw.tile([P, RR + 1, WP], F16, name="t", tag=f"w{ki}", bufs=1)
        nc.scalar.activation(out=wk[:, 0:RR, 1:1 + W], in_=d[:, 0], func=AF.Exp,
                             bias=LNG[ki], scale=scale)
        Wt[ki] = wk

    # --- W halo fixups (mirror taps read shifted views) ---
    # col halos: W00 right <- W02 col x=510; W02 left <- W00 col x=1; W10 right <- own x=511
    nc.gpsimd.tensor_copy(out=Wt[0][:, 0:RR, WP - 1:WP], in_=Wt[2][:, 0:RR, W - 1:W])
    nc.gpsimd.tensor_copy(out=Wt[2][:, 0:RR, 0:1], in_=Wt[0][:, 0:RR, 2:3])
    nc.gpsimd.tensor_copy(out=Wt[3][:, 0:RR, WP - 1:WP], in_=Wt[3][:, 0:RR, WP - 2:WP - 1])
    # bottom halo rows (taps with mirror dy-shift): interior partitions
    for k in (0, 1, 2):
        nc.sync.dma_start(out=Wt[k][0:P - 1, RR:RR + 1, :], in_=Wt[k][1:P, 0:1, :])
    # last partition bottom rows via the reflect partner plane
    nc.gpsimd.dma_start(out=Wt[0][P - 1:P, RR:RR + 1, 2:WP],
                        in_=Wt[2][P - 1:P, RR - 1:RR, 1:WP - 1])
    nc.gpsimd.dma_start(out=Wt[1][P - 1:P, RR:RR + 1, 1:WP - 1],
                        in_=Wt[1][P - 1:P, RR - 1:RR, 1:WP - 1])
    nc.gpsimd.dma_start(out=Wt[2][P - 1:P, RR:RR + 1, 0:W],
                        in_=Wt[0][P - 1:P, RR - 1:RR, 1:WP - 1])

    # --- per-row products + PE accumulation + normalize ---
    for r in range(RR):
        pr = pprod.tile([P, 8, NCH, W], F16, name="t", tag="prod", bufs=1)
        for ki, (dy, dx) in enumerate(TAPS):
            wQ = Wt[ki][:, r:r + 1, 1:1 + W].broadcast_to((P, NCH, W))
            nc.vector.tensor_tensor(pr[:, ki], Xin[:, :, dy + r, dx:dx + W], wQ,
                                    op=OP.mult)
        for ki, (dy, dx) in enumerate(TAPS):
            sy, sx = 1 - dy, 1 - dx
            if ki == 0:
                # split off the last column: it reads W00's right halo (from
                # W02), which would stall the whole mul on tap 2's exp
                wZ = Wt[ki][:, r + sy:r + sy + 1, 2:1 + W].broadcast_to((P, NCH, W - 1))
                nc.vector.tensor_tensor(pr[:, 4 + ki, :, 0:W - 1],
                                        Xin[:, :, r + sy + 1, 2:1 + W], wZ, op=OP.mult)
                wZl = Wt[ki][:, r + sy:r + sy + 1, 1 + W:2 + W].broadcast_to((P, NCH, 1))
                nc.vector.tensor_tensor(pr[:, 4 + ki, :, W - 1:W],
                                        Xin[:, :, r + sy + 1, 1 + W:2 + W], wZl, op=OP.mult)
                continue
            wZ = Wt[ki][:, r + sy:r + sy + 1, 1 + sx:1 + sx + W].broadcast_to((P, NCH, W))
            nc.vector.tensor_tensor(pr[:, 4 + ki], Xin[:, :, r + sy + 1, 1 + sx:1 + sx + W],
                                    wZ, op=OP.mult)
        dn = ppsum.tile([P, W], F32, name="t", tag="psd", bufs=3)
        nc.tensor.matmul(dn[:], gid[:], ones[:], start=True, stop=False)
        for ki, (dy, dx) in enumerate(TAPS):
            sy, sx = 1 - dy, 1 - dx
            nc.tensor.matmul(dn[:], idt[:], Wt[ki][:, r, 1:1 + W],
                             start=False, stop=False)
            nc.tensor.matmul(dn[:], idt[:], Wt[ki][:, r + sy, 1 + sx:1 + sx + W],
                             start=False, stop=(ki == 3))
        rec = psmall.tile([P, W], F32, name="t", tag="rec", bufs=3)
        nc.vector.reciprocal_approx_fast(out=rec[:], in_=dn[:])
        for c in range(NCH):
            t = ppsum.tile([P, W], F32, name="t", tag="pst", bufs=5)
            nc.tensor.matmul(t[:], gid[:], Xin[:, c, r + 1, 1:1 + W],
                             start=True, stop=False)
            for ki in range(4):
                nc.tensor.matmul(t[:], idt[:], pr[:, ki, c], start=False, stop=False)
                nc.tensor.matmul(t[:], idt[:], pr[:, 4 + ki, c],
                                 start=False, stop=(ki == 3))
            nc.vector.tensor_tensor(Xout[:, c, r + 1, 1:1 + W], t[:], rec[:], op=OP.mult)
    _halo_x(nc, Xout)


def _median(nc, pools, Xin, Xout, yview=None):
    """3x3 median: row-phase (lo/med/hi over 3-row windows, consuming Xin's
    halos) then col-phase on free-dim shifted views. If yview is given the
    result rows are DMAed there instead of written to Xout."""
    px, pw, pd, pprod, ppsum, psmall = pools
    mn = lambda o, a, b: nc.vector.tensor_tensor(o, a, b, op=OP.min)
    mx = lambda o, a, b: nc.vector.tensor_max(o, a, b)
    for r in range(RR):
        R0, R1, R2 = Xin[:, :, r], Xin[:, :, r + 1], Xin[:, :, r + 2]
        t1 = psmall.tile([P, NCH, WP], F16, name="t", tag=f"m1{r % 2}", bufs=1)
        t2 = psmall.tile([P, NCH, WP], F16, name="t", tag=f"m2{r % 2}", bufs=1)
        lo = psmall.tile([P, NCH, WP], F16, name="t", tag="m3", bufs=1)
        hi = psmall.tile([P, NCH, WP], F16, name="t", tag="m4", bufs=1)
        mn(t1[:], R0, R1)
        mx(t2[:], R0, R1)
        mn(lo[:], t1[:], R2)
        mx(hi[:], t2[:], R2)
        mn(t2[:], t2[:], R2)          # med partial
        mx(t1[:], t1[:], t2[:])       # t1 = med3 of rows
        m = t1
        AS, BS, CS = slice(0, W), slice(1, 1 + W), slice(2, 2 + W)
        H = psmall.tile([P, NCH, W], F16, name="t", tag="n1", bufs=1)
        L = psmall.tile([P, NCH, W], F16, name="t", tag="n2", bufs=1)
        u1 = psmall.tile([P, NCH, W], F16, name="t", tag="n3", bufs=1)
        u2 = psmall.tile([P, NCH, W], F16, name="t", tag="n4", bufs=1)
        v1 = psmall.tile([P, NCH, W], F16, name="t", tag="n4", bufs=1)
        mn(H[:], hi[:, :, AS], hi[:, :, BS])
        mn(H[:], H[:], hi[:, :, CS])
        mx(L[:], lo[:, :, AS], lo[:, :, BS])
        mx(L[:], L[:], lo[:, :, CS])
        mn(u1[:], m[:, :, AS], m[:, :, BS])
        mx(u2[:], m[:, :, AS], m[:, :, BS])
        mn(u2[:], u2[:], m[:, :, CS])
        mx(u1[:], u1[:], u2[:])       # u1 = M
        mn(v1[:], H[:], u1[:])
        mx(H[:], H[:], u1[:])
        mn(H[:], H[:], L[:])
        if yview is not None:
            orow = psmall.tile([P, NCH, W], F16, name="t", tag="or", bufs=1)
            mx(orow[:], v1[:], H[:])
            nc.sync.dma_start(out=yview[:, :, r:r + 1, :], in_=orow[:].unsqueeze(2))
        else:
            mx(Xout[:, :, r + 1, 1:1 + W], v1[:], H[:])
    if yview is None:
        _halo_x(nc, Xout)


def build():
    nc = bacc.Bacc("TRN2", target_bir_lowering=False, debug=False)
    _register_consts(nc)
    xin = nc.dram_tensor("xin", [NIMG, P, NCH, H6, WP], F16, kind="ExternalInput").ap()
    idg = nc.dram_tensor("idg", [2, P, P], F16, kind="ExternalInput").ap()
    yout = nc.dram_tensor("yout", [NIMG, P, NCH, RR, W], F16, kind="ExternalOutput").ap()

    with tile.TileContext(nc) as tc, ExitStack() as ctx:
        px = ctx.enter_context(tc.tile_pool(name="px", bufs=1))
        pw = ctx.enter_context(tc.tile_pool(name="pw", bufs=1))
        pd = ctx.enter_context(tc.tile_pool(name="pd", bufs=1))
        pprod = ctx.enter_context(tc.tile_pool(name="pprod", bufs=1))
        ppsum = ctx.enter_context(tc.psum_pool(name="ppsum", bufs=1))
        psmall = ctx.enter_context(tc.tile_pool(name="psmall", bufs=1))
        pools = (px, pw, pd, pprod, ppsum, psmall)

        idt = psmall.tile([P, P], F16, name="t", tag="id", bufs=1)
        nc.sync.dma_start(out=idt[:], in_=idg[0])
        gid = psmall.tile([P, P], F16, name="t", tag="gid", bufs=1)
        nc.sync.dma_start(out=gid[:], in_=idg[1])
        ones = psmall.tile([P, W], F16, name="t", tag="ones", bufs=1)
        nc.gpsimd.memset(ones[:], 1.0)

        X = []
        for img in range(NIMG):
            xt = px.tile([P, NCH, H6, WP], F16, name="t", tag="x", bufs=4)
            nc.gpsimd.dma_start(out=xt[:], in_=xin[img])
            X.append(xt)
        for img in range(NIMG):
            o = px.tile([P, NCH, H6, WP], F16, name="t", tag="x", bufs=4)
            _bilateral(nc, pools, X[img], o, SCALE1, idt, gid, ones)
            X[img] = o
        o = px.tile([P, NCH, H6, WP], F16, name="t", tag="x", bufs=4)
        _bilateral(nc, pools, X[0], o, SCALE2, idt, gid, ones)
        X[0] = o
        o = px.tile([P, NCH, H6, WP], F16, name="t", tag="x", bufs=4)
        _median(nc, pools, X[0], o)
        m0 = o
        o = px.tile([P, NCH, H6, WP], F16, name="t", tag="x", bufs=4)
        _bilateral(nc, pools, X[1], o, SCALE2, idt, gid, ones)
        X[1] = o
        _median(nc, pools, m0, None, yview=yout[0])
        o = px.tile([P, NCH, H6, WP], F16, name="t", tag="x", bufs=4)
        _median(nc, pools, X[1], o)
        X[1] = o
        _median(nc, pools, X[1], None, yview=yout[1])

    nc.compile()
    return nc


_NC_CACHE = None


def _get_nc():
    global _NC_CACHE
    if _NC_CACHE is None:
        _NC_CACHE = build()
    return _NC_CACHE


def _prep_inputs(x):
    xpad = np.pad(x, ((0, 0), (0, 0), (1, 1), (1, 1)), mode="reflect")
    rows = np.arange(P)[:, None] * RR + np.arange(H6)[None, :]
    win = xpad[:, :, rows, :]                          # (16,3,128,6,514)
    win = win.transpose(0, 2, 1, 3, 4).astype(np.float16)
    return np.ascontiguousarray(win.reshape(N_CORES, NIMG, P, NCH, H6, WP))


def kernel(x):
    x = np.ascontiguousarray(np.asarray(x), dtype=np.float32)
    assert x.shape == (16, 3, 512, 512)
    nc = _get_nc()
    xin = _prep_inputs(x)
    idg = np.ascontiguousarray(
        np.stack([np.eye(P), GC * np.eye(P)]).astype(np.float16))
    in_maps = [{"xin": xin[c], "idg": idg} for c in range(N_CORES)]
    res = run_bass_kernel_spmd(nc, in_maps, list(range(N_CORES)))
    out = np.empty((16, 3, 512, 512), np.float32)
    for c in range(N_CORES):
        y = res.results[c]["yout"]                     # (2,128,3,4,512) f16
        out[2 * c:2 * c + 2] = (y.transpose(0, 2, 1, 3, 4)
                                 .reshape(NIMG, NCH, 512, 512).astype(np.float32))
    return out


# revision 40
# speedup vs baseline: 1.0879x; 1.0089x over previous
"""Trainium2 Bass kernel for Bil_layer: 2x bilateral(3x3) + 2x median(3x3).

Data parallel: 2 images x 3 channels per core across 8 cores. Layout per
512x512 plane: 128 partitions x 4 rows; padded tile [128, 3ch, 6, 514]
holds rows -1..4 and cols -1..512 (reflect halos).

Bilateral uses the mirror-weight identity w_{2-dy,2-dx}(y,x) =
w_{dy,dx}(y+dy',x+dx') (dy'=1-dy, dx'=1-dx), so only 4 of 8 tap weights
are computed; mirrored taps read shifted views. Weighted sums accumulate
in PSUM fp32 via PE identity matmuls. Median runs row-phase first
(consuming the input tile's halos directly), then column-phase on free-dim
shifted views -- no intermediate halo exchange.
"""
import numpy as np
from contextlib import ExitStack

import concourse.tile as tile
from concourse import bacc, mybir
from concourse.bass_utils import run_bass_kernel_spmd

P = 128
RR = 4            # data rows per partition
H6 = 6            # padded rows (-1..4)
W = 512
WP = 514          # padded cols (-1..512)
NCH = 3
NIMG = 2
N_CORES = 8

SIGMA_COLOR = 0.1
COLOR2 = 0.01
SIGMA_SPACE = 10.0

F16 = mybir.dt.float16
F32 = mybir.dt.float32
OP = mybir.AluOpType
AF = mybir.ActivationFunctionType


def _gauss1():
    ax = np.arange(3, dtype=np.float64) - 1.0
    g = np.exp(-0.5 * (ax / SIGMA_SPACE) ** 2)
    return g / g.sum()


GO = _gauss1()
GC = float(GO[1] * GO[1])                 # center weight
LNG_C = float(np.log(GO[0] * GO[0]))      # ln g for taps (0,0),(0,2)
LNG_E = float(np.log(GO[0] * GO[1]))      # ln g for taps (0,1),(1,0)
TAPS = [(0, 0), (0, 1), (0, 2), (1, 0)]   # first-half taps; mirror shift = (1-dy, 1-dx)
LNG = [LNG_C, LNG_E, LNG_C, LNG_E]
SCALE1 = float(-0.5 / SIGMA_COLOR ** 2)
SCALE2 = float(-0.5 / COLOR2 ** 2)


def _register_consts(nc):
    for v in sorted({LNG_C, LNG_E}):
        if (F32, v) in nc.const_aps.aps:
            continue
        t = nc.alloc_sbuf_tensor(f"const-f32-{abs(hash(v))}", [P, 1], F32)
        nc.gpsimd.memset(t.ap(), v)
        nc.const_aps.aps[(F32, v)] = t.ap()
    nc.all_engine_barrier()


def _halo_x(nc, X):
    """Fill halos of padded tile X [P, NCH, 6, 514] whose interior
    (rows 1..4, cols 1..512) was written. Col reflect copies first, then
    full-width row halos (so corners ride along)."""
    nc.gpsimd.tensor_copy(out=X[:, :, 1:5, 0:1], in_=X[:, :, 1:5, 2:3])
    nc.gpsimd.tensor_copy(out=X[:, :, 1:5, WP - 1:WP], in_=X[:, :, 1:5, WP - 3:WP - 2])
    nc.gpsimd.dma_start(out=X[0:1, :, 0:1, :], in_=X[0:1, :, 2:3, :])
    nc.gpsimd.dma_start(out=X[P - 1:P, :, 5:6, :], in_=X[P - 1:P, :, 3:4, :])
    nc.sync.dma_start(out=X[1:P, :, 0:1, :], in_=X[0:P - 1, :, 4:5, :])
    nc.sync.dma_start(out=X[0:P - 1, :, 5:6, :], in_=X[1:P, :, 1:2, :])


def _bilateral(nc, pools, Xin, Xout, scale, idt, gid, ones):
    px, pw, pd, pprod, ppsum, psmall = pools

    # --- per-tap color weights (4 first-half taps) ---
    Wt = [None] * 4
    for emi, ki in enumerate((0, 1, 2, 3)):
        dy, dx = TAPS[ki]
        d = pd.tile([P, NCH, RR, W], F16, name="t", tag=f"d{ki}", bufs=1)
        eng = nc.vector if emi < 2 else nc.gpsimd
        eng.tensor_tensor(d[:], Xin[:, :, dy:dy + RR, dx:dx + W],
                          Xin[:, :, 1:5, 1:1 + W], op=OP.subtract)
        nc.scalar.activation(out=d[:], in_=d[:], func=AF.Abs)
        eng.tensor_tensor(d[:, 0], d[:, 0], d[:, 1], op=OP.add)
        eng.tensor_tensor(d[:, 0], d[:, 0], d[:, 2], op=OP.add)
        nc.scalar.activation(out=d[:, 0], in_=d[:, 0], func=AF.Square)
        wk = pw.tile([P, RR + 1, WP], F16, name="t", tag=f"w{ki}", bufs=1)
        nc.scalar.activation(out=wk[:, 0:RR, 1:1 + W], in_=d[:, 0], func=AF.Exp,
                             bias=LNG[ki], scale=scale)
        Wt[ki] = wk

    # --- W halo fixups (mirror taps read shifted views) ---
    # col halos: W00 right <- W02 col x=510; W02 left <- W00 col x=1; W10 right <- own x=511
    nc.gpsimd.tensor_copy(out=Wt[0][:, 0:RR, WP - 1:WP], in_=Wt[2][:, 0:RR, W - 1:W])
    nc.gpsimd.tensor_copy(out=Wt[2][:, 0:RR, 0:1], in_=Wt[0][:, 0:RR, 2:3])
    nc.gpsimd.tensor_copy(out=Wt[3][:, 0:RR, WP - 1:WP], in_=Wt[3][:, 0:RR, WP - 2:WP - 1])
    # bottom halo rows (taps with mirror dy-shift): interior partitions
    for k in (0, 1, 2):
        nc.sync.dma_start(out=Wt[k][0:P - 1, RR:RR + 1, :], in_=Wt[k][1:P, 0:1, :])
    # last partition bottom rows via the reflect partner plane
    nc.gpsimd.dma_start(out=Wt[0][P - 1:P, RR:RR + 1, 2:WP],
                        in_=Wt[2][P - 1:P, RR - 1:RR, 1:WP - 1])
    nc.gpsimd.dma_start(out=Wt[1][P - 1:P, RR:RR + 1, 1:WP - 1],
                        in_=Wt[1][P - 1:P, RR - 1:RR, 1:WP - 1])
    nc.gpsimd.dma_start(out=Wt[2][P - 1:P, RR:RR + 1, 0:W],
                        in_=Wt[0][P - 1:P, RR - 1:RR, 1:WP - 1])

    # --- per-row products + PE accumulation + normalize ---
    for r in range(RR):
        pr = pprod.tile([P, 8, NCH, W], F16, name="t", tag="prod", bufs=1)
        for ki, (dy, dx) in enumerate(TAPS):
            wQ = Wt[ki][:, r:r + 1, 1:1 + W].broadcast_to((P, NCH, W))
            nc.vector.tensor_tensor(pr[:, ki], Xin[:, :, dy + r, dx:dx + W], wQ,
                                    op=OP.mult)
        for ki, (dy, dx) in enumerate(TAPS):
            sy, sx = 1 - dy, 1 - dx
            if ki == 0:
                # split off the last column: it reads W00's right halo (from
                # W02), which would stall the whole mul on tap 2's exp
                wZ = Wt[ki][:, r + sy:r + sy + 1, 2:1 + W].broadcast_to((P, NCH, W - 1))
                nc.vector.tensor_tensor(pr[:, 4 + ki, :, 0:W - 1],
                                        Xin[:, :, r + sy + 1, 2:1 + W], wZ, op=OP.mult)
                wZl = Wt[ki][:, r + sy:r + sy + 1, 1 + W:2 + W].broadcast_to((P, NCH, 1))
                nc.vector.tensor_tensor(pr[:, 4 + ki, :, W - 1:W],
                                        Xin[:, :, r + sy + 1, 1 + W:2 + W], wZl, op=OP.mult)
                continue
            wZ = Wt[ki][:, r + sy:r + sy + 1, 1 + sx:1 + sx + W].broadcast_to((P, NCH, W))
            nc.vector.tensor_tensor(pr[:, 4 + ki], Xin[:, :, r + sy + 1, 1 + sx:1 + sx + W],
                                    wZ, op=OP.mult)
        dn = ppsum.tile([P, W], F32, name="t", tag="psd", bufs=3)
        nc.tensor.matmul(dn[:], gid[:], ones[:], start=True, stop=False)
        for ki, (dy, dx) in enumerate(TAPS):
            sy, sx = 1 - dy, 1 - dx
            nc.tensor.matmul(dn[:], idt[:], Wt[ki][:, r, 1:1 + W],
                             start=False, stop=False)
            nc.tensor.matmul(dn[:], idt[:], Wt[ki][:, r + sy, 1 + sx:1 + sx + W],
                             start=False, stop=(ki == 3))
        rec = psmall.tile([P, W], F32, name="t", tag="rec", bufs=3)
        nc.vector.reciprocal_approx_fast(out=rec[:], in_=dn[:])
        for c in range(NCH):
            t = ppsum.tile([P, W], F32, name="t", tag="pst", bufs=5)
            nc.tensor.matmul(t[:], gid[:], Xin[:, c, r + 1, 1:1 + W],
                             start=True, stop=False)
            for ki in range(4):
                nc.tensor.matmul(t[:], idt[:], pr[:, ki, c], start=False, stop=False)
                nc.tensor.matmul(t[:], idt[:], pr[:, 4 + ki, c],
                                 start=False, stop=(ki == 3))
            nc.vector.tensor_tensor(Xout[:, c, r + 1, 1:1 + W], t[:], rec[:], op=OP.mult)
    _halo_x(nc, Xout)


def _median(nc, pools, Xin, Xout, yview=None):
    """3x3 median: row-phase (lo/med/hi over 3-row windows, consuming Xin's
    halos) then col-phase on free-dim shifted views. If yview is given the
    result rows are DMAed there instead of written to Xout."""
    px, pw, pd, pprod, ppsum, psmall = pools
    mn = lambda o, a, b: nc.vector.tensor_tensor(o, a, b, op=OP.min)
    mx = lambda o, a, b: nc.vector.tensor_max(o, a, b)
    for r in range(RR):
        R0, R1, R2 = Xin[:, :, r], Xin[:, :, r + 1], Xin[:, :, r + 2]
        t1 = psmall.tile([P, NCH, WP], F16, name="t", tag=f"m1{r % 2}", bufs=1)
        t2 = psmall.tile([P, NCH, WP], F16, name="t", tag=f"m2{r % 2}", bufs=1)
        lo = psmall.tile([P, NCH, WP], F16, name="t", tag="m3", bufs=1)
        hi = psmall.tile([P, NCH, WP], F16, name="t", tag="m4", bufs=1)
        mn(t1[:], R0, R1)
        mx(t2[:], R0, R1)
        mn(lo[:], t1[:], R2)
        mx(hi[:], t2[:], R2)
        mn(t2[:], t2[:], R2)          # med partial
        mx(t1[:], t1[:], t2[:])       # t1 = med3 of rows
        m = t1
        AS, BS, CS = slice(0, W), slice(1, 1 + W), slice(2, 2 + W)
        H = psmall.tile([P, NCH, W], F16, name="t", tag="n1", bufs=1)
        L = psmall.tile([P, NCH, W], F16, name="t", tag="n2", bufs=1)
        u1 = psmall.tile([P, NCH, W], F16, name="t", tag="n3", bufs=1)
        u2 = psmall.tile([P, NCH, W], F16, name="t", tag="n4", bufs=1)
        v1 = psmall.tile([P, NCH, W], F16, name="t", tag="n4", bufs=1)
        mn(H[:], hi[:, :, AS], hi[:, :, BS])
        mn(H[:], H[:], hi[:, :, CS])
        mx(L[:], lo[:, :, AS], lo[:, :, BS])
        mx(L[:], L[:], lo[:, :, CS])
        mn(u1[:], m[:, :, AS], m[:, :, BS])
        mx(u2[:], m[:, :, AS], m[:, :, BS])
        mn(u2[:], u2[:], m[:, :, CS])
        mx(u1[:], u1[:], u2[:])       # u1 = M
        mn(v1[:], H[:], u1[:])
        mx(H[:], H[:], u1[:])
        mn(H[:], H[:], L[:])
        if yview is not None:
            orow = psmall.tile([P, NCH, W], F16, name="t", tag="or", bufs=1)
            mx(orow[:], v1[:], H[:])
            nc.sync.dma_start(out=yview[:, :, r:r + 1, :], in_=orow[:].unsqueeze(2))
        else:
            mx(Xout[:, :, r + 1, 1:1 + W], v1[:], H[:])
    if yview is None:
        _halo_x(nc, Xout)


def build():
    nc = bacc.Bacc("TRN2", target_bir_lowering=False, debug=False)
    _register_consts(nc)
    xin = nc.dram_tensor("xin", [NIMG, P, NCH, H6, WP], F16, kind="ExternalInput").ap()
    idg = nc.dram_tensor("idg", [2, P, P], F16, kind="ExternalInput").ap()
    yout = nc.dram_tensor("yout", [NIMG, P, NCH, RR, W], F16, kind="ExternalOutput").ap()

    with tile.TileContext(nc) as tc, ExitStack() as ctx:
        px = ctx.enter_context(tc.tile_pool(name="px", bufs=1))
        pw = ctx.enter_context(tc.tile_pool(name="pw", bufs=1))
        pd = ctx.enter_context(tc.tile_pool(name="pd", bufs=1))
        pprod = ctx.enter_context(tc.tile_pool(name="pprod", bufs=1))
        ppsum = ctx.enter_context(tc.psum_pool(name="ppsum", bufs=1))
        psmall = ctx.enter_context(tc.tile_pool(name="psmall", bufs=1))
        pools = (px, pw, pd, pprod, ppsum, psmall)

        idt = psmall.tile([P, P], F16, name="t", tag="id", bufs=1)
        nc.sync.dma_start(out=idt[:], in_=idg[0])
        gid = psmall.tile([P, P], F16, name="t", tag="gid", bufs=1)
        nc.sync.dma_start(out=gid[:], in_=idg[1])
        ones = psmall.tile([P, W], F16, name="t", tag="ones", bufs=1)
        nc.gpsimd.memset(ones[:], 1.0)

        X = []
        for img in range(NIMG):
            xt = px.tile([P, NCH, H6, WP], F16, name="t", tag="x", bufs=4)
            nc.gpsimd.dma_start(out=xt[:], in_=xin[img])
            X.append(xt)
        for img in range(NIMG):
            o = px.tile([P, NCH, H6, WP], F16, name="t", tag="x", bufs=4)
            _bilateral(nc, pools, X[img], o, SCALE1, idt, gid, ones)
            X[img] = o
        o = px.tile([P, NCH, H6, WP], F16, name="t", tag="x", bufs=4)
        _bilateral(nc, pools, X[0], o, SCALE2, idt, gid, ones)
        X[0] = o
        o = px.tile([P, NCH, H6, WP], F16, name="t", tag="x", bufs=4)
        _median(nc, pools, X[0], o)
        m0 = o
        o = px.tile([P, NCH, H6, WP], F16, name="t", tag="x", bufs=4)
        _bilateral(nc, pools, X[1], o, SCALE2, idt, gid, ones)
        X[1] = o
        _median(nc, pools, m0, None, yview=yout[0])
        o = px.tile([P, NCH, H6, WP], F16, name="t", tag="x", bufs=4)
        _median(nc, pools, X[1], o)
        X[1] = o
        _median(nc, pools, X[1], None, yview=yout[1])

    nc.compile()
    return nc


_NC_CACHE = None


def _get_nc():
    global _NC_CACHE
    if _NC_CACHE is None:
        _NC_CACHE = build()
    return _NC_CACHE


def _prep_inputs(x):
    xpad = np.pad(x, ((0, 0), (0, 0), (1, 1), (1, 1)), mode="reflect")
    rows = np.arange(P)[:, None] * RR + np.arange(H6)[None, :]
    win = xpad[:, :, rows, :]                          # (16,3,128,6,514)
    win = win.transpose(0, 2, 1, 3, 4).astype(np.float16)
    return np.ascontiguousarray(win.reshape(N_CORES, NIMG, P, NCH, H6, WP))


def kernel(x):
    x = np.ascontiguousarray(np.asarray(x), dtype=np.float32)
    assert x.shape == (16, 3, 512, 512)
    nc = _get_nc()
    xin = _prep_inputs(x)
    idg = np.ascontiguousarray(
        np.stack([np.eye(P), GC * np.eye(P)]).astype(np.float16))
    in_maps = [{"xin": xin[c], "idg": idg} for c in range(N_CORES)]
    res = run_bass_kernel_spmd(nc, in_maps, list(range(N_CORES)))
    out = np.empty((16, 3, 512, 512), np.float32)
    for c in range(N_CORES):
        y = res.results[c]["yout"]                     # (2,128,3,4,512) f16
        out[2 * c:2 * c + 2] = (y.transpose(0, 2, 1, 3, 4)
                                 .reshape(NIMG, NCH, 512, 512).astype(np.float32))
    return out
